# revision 3
# baseline (speedup 1.0000x reference)
# Dilated causal self-attention kernel for Trainium2 (8 NeuronCores).
#
# Reference computation (see problem):
#   x (4, 8192, 1024) -> reshape (4, 4, 2048, 1024) -> take every 4th token
#   -> per-segment causal MHA (16 heads, dh=64) -> scatter back into zeros.
#
# Sharding: 16 independent (batch, segment) attention problems, 2 per core.
# Host does the dilated gather + transpose + bf16 cast and the final scatter
# into the zero background; each core runs QKV -> per-head causal softmax
# attention -> output projection on its 2 segments.
#
# Device layout (all feature-major where possible):
#   xiT    [C, M]  (per segment)         - input, bf16
#   qkT    [2C, M] feature-major         - q rows pre-scaled by 1/sqrt(dh)
#                                          (folded into w_in on host)
#   v      [M, C]  token-major           - v bias folds into output bias
#   scores [128 q, n k] per (head, q-chunk), n = (qc+1)*128 (causal skip)
#   p = exp(scores) (no max subtraction: scores ~ N(0,1)), accum_out = denom
#   PV: outT[dh, M] = sum_kc v_kc^T @ pT_kc   (pT via PE transpose)
#   yT = w_outT^T @ oT + b_out_eff  -> DMA out feature-major

import sys

sys.path.insert(0, "/opt/trn_rl_repo")

import numpy as np
import ml_dtypes

import concourse.bacc as bacc
import concourse.mybir as mybir
from concourse.tile import TileContext
from concourse.bass_utils import run_bass_kernel_spmd
from concourse.masks import make_identity

BF16 = ml_dtypes.bfloat16

B, N, C = 4, 8192, 1024
W_SEG, RATE, H = 2048, 4, 16
DH = C // H            # 64
S = N // W_SEG         # 4 segments per batch
M = W_SEG // RATE      # 512 tokens per segment
N_CORES = 8
SEG_PER_CORE = (B * S) // N_CORES  # 2

FP32 = mybir.dt.float32
BF = mybir.dt.bfloat16

_CACHE = {}


def _build():
    nc = bacc.Bacc()
    xiT = nc.dram_tensor("xiT", [SEG_PER_CORE * C, M], BF, kind="ExternalInput")
    w_inT = nc.dram_tensor("w_inT", [C, 3 * C], BF, kind="ExternalInput")
    w_outT = nc.dram_tensor("w_outT", [C, C], BF, kind="ExternalInput")
    bqk = nc.dram_tensor("bqk", [128, 16], FP32, kind="ExternalInput")
    bout = nc.dram_tensor("bout", [128, 8], FP32, kind="ExternalInput")
    tri = nc.dram_tensor("tri", [128, 128], FP32, kind="ExternalInput")
    yT = nc.dram_tensor("yT", [SEG_PER_CORE * C, M], FP32, kind="ExternalOutput")

    CT = C // 128  # 8 contraction chunks

    from contextlib import ExitStack
    with TileContext(nc) as tc, ExitStack() as ctx:
        consts = ctx.enter_context(tc.tile_pool(name="consts", bufs=1))
        wpool = ctx.enter_context(tc.tile_pool(name="weights", bufs=1))
        xpool = ctx.enter_context(tc.tile_pool(name="x", bufs=2 * CT))
        qkpool = ctx.enter_context(tc.tile_pool(name="qk", bufs=16))
        vpool = ctx.enter_context(tc.tile_pool(name="v", bufs=8))
        ppool = ctx.enter_context(tc.tile_pool(name="p", bufs=6))
        ptpool = ctx.enter_context(tc.tile_pool(name="pt", bufs=8))
        otpool = ctx.enter_context(tc.tile_pool(name="ot", bufs=8))
        ypool = ctx.enter_context(tc.tile_pool(name="y", bufs=3))
        small = ctx.enter_context(tc.tile_pool(name="small", bufs=8))
        psA = ctx.enter_context(tc.tile_pool(name="psA", bufs=2, space="PSUM"))
        psS = ctx.enter_context(tc.tile_pool(name="psS", bufs=2, space="PSUM"))
        psT = ctx.enter_context(tc.tile_pool(name="psT", bufs=2, space="PSUM"))
        psO = ctx.enter_context(tc.tile_pool(name="psO", bufs=2, space="PSUM"))

        if True:
            ident = consts.tile([128, 128], BF, tag="ident")
            make_identity(nc, ident[:])
            tri_sb = consts.tile([128, 128], FP32, tag="tri")
            nc.sync.dma_start(out=tri_sb[:], in_=tri[:, :])
            bqk_sb = consts.tile([128, 16], FP32, tag="bqk")
            nc.sync.dma_start(out=bqk_sb[:], in_=bqk[:, :])
            bout_sb = consts.tile([128, 8], FP32, tag="bout")
            nc.sync.dma_start(out=bout_sb[:], in_=bout[:, :])

            w_in_sb = []
            for ct in range(CT):
                t = wpool.tile([128, 3 * C], BF, tag=f"w_in{ct}")
                nc.sync.dma_start(out=t[:], in_=w_inT[ct * 128:(ct + 1) * 128, :])
                w_in_sb.append(t)
            w_out_sb = []
            for ct in range(CT):
                t = wpool.tile([128, C], BF, tag=f"w_out{ct}")
                nc.sync.dma_start(out=t[:], in_=w_outT[ct * 128:(ct + 1) * 128, :])
                w_out_sb.append(t)

            for seg in range(SEG_PER_CORE):
                base = seg * C
                x_sb = []
                for ct in range(CT):
                    t = xpool.tile([128, M], BF, tag="x")
                    nc.sync.dma_start(
                        out=t[:], in_=xiT[base + ct * 128:base + (ct + 1) * 128, :])
                    x_sb.append(t)

                # --- QKV (q, k): feature-major, interleaved q/k so heads
                # can start as soon as their q and k tiles are ready.
                qk_sb = [None] * 16
                for pair in range(8):
                    for half in (0, 8):
                        et = pair + half
                        ps = psA.tile([128, M], FP32, tag="psA")
                        for ct in range(CT):
                            nc.tensor.matmul(
                                ps[:],
                                lhsT=w_in_sb[ct][:, et * 128:(et + 1) * 128],
                                rhs=x_sb[ct][:],
                                start=(ct == 0), stop=(ct == CT - 1))
                        t = qkpool.tile([128, M], BF, tag="qk")
                        nc.scalar.activation(
                            out=t[:], in_=ps[:],
                            func=mybir.ActivationFunctionType.Identity,
                            bias=bqk_sb[:, et:et + 1], scale=1.0)
                        qk_sb[et] = t

                # --- QKV (v): token-major [M, C]
                v_sb = []
                for tt in range(4):
                    vt = vpool.tile([128, C], BF, tag="v")
                    for nf in range(2):
                        ps = psA.tile([128, M], FP32, tag="psA")
                        for ct in range(CT):
                            nc.tensor.matmul(
                                ps[:],
                                lhsT=x_sb[ct][:, tt * 128:(tt + 1) * 128],
                                rhs=w_in_sb[ct][:, 2 * C + nf * 512:2 * C + (nf + 1) * 512],
                                start=(ct == 0), stop=(ct == CT - 1))
                        nc.scalar.copy(out=vt[:, nf * 512:(nf + 1) * 512], in_=ps[:])
                    v_sb.append(vt)

                # --- attention output, feature-major oT [C, M]
                oT_sb = [otpool.tile([128, M], BF, tag="ot", name="ot") for _ in range(8)]

                for h in range(H):
                    et, row = h // 2, (h % 2) * 64
                    qh = qk_sb[et][row:row + 64, :]
                    kh = qk_sb[8 + et][row:row + 64, :]
                    pt_sb = [ptpool.tile([128, M], BF, tag="pt", name="pt") for _ in range(4)]
                    for qc in range(4):
                        n = (qc + 1) * 128
                        ps = psS.tile([128, M], FP32, tag="psS")
                        nc.tensor.matmul(
                            ps[:, :n],
                            lhsT=qh[:, qc * 128:(qc + 1) * 128],
                            rhs=kh[:, :n], start=True, stop=True)
                        # causal mask on the diagonal block
                        nc.vector.tensor_add(
                            ps[:, qc * 128:n], ps[:, qc * 128:n], tri_sb[:])
                        p = ppool.tile([128, M], BF, tag="p")
                        den = small.tile([128, 1], FP32, tag="den")
                        nc.scalar.activation(
                            out=p[:, :n], in_=ps[:, :n],
                            func=mybir.ActivationFunctionType.Exp,
                            accum_out=den[:])
                        rden = small.tile([128, 1], FP32, tag="rden")
                        nc.vector.reciprocal(rden[:], den[:])
                        nc.vector.tensor_scalar_mul(p[:, :n], p[:, :n], rden[:])
                        for kc in range(qc + 1):
                            pst = psT.tile([128, 128], BF, tag="psT")
                            nc.tensor.transpose(
                                pst[:], p[:, kc * 128:(kc + 1) * 128], ident[:])
                            nc.vector.tensor_copy(
                                out=pt_sb[kc][:, qc * 128:(qc + 1) * 128],
                                in_=pst[:])
                    po = psO.tile([64, M], FP32, tag="psO")
                    for kc in range(4):
                        nc.tensor.matmul(
                            po[:, kc * 128:],
                            lhsT=v_sb[kc][:, h * 64:(h + 1) * 64],
                            rhs=pt_sb[kc][:, kc * 128:],
                            start=(kc == 0), stop=(kc == 3))
                    nc.scalar.copy(out=oT_sb[h // 2][row:row + 64, :], in_=po[:])

                # --- output projection, feature-major yT [C, M]
                for ot in range(8):
                    ps = psA.tile([128, M], FP32, tag="psA")
                    for ct in range(CT):
                        nc.tensor.matmul(
                            ps[:],
                            lhsT=w_out_sb[ct][:, ot * 128:(ot + 1) * 128],
                            rhs=oT_sb[ct][:],
                            start=(ct == 0), stop=(ct == CT - 1))
                    yt = ypool.tile([128, M], FP32, tag="y")
                    nc.scalar.activation(
                        out=yt[:], in_=ps[:],
                        func=mybir.ActivationFunctionType.Identity,
                        bias=bout_sb[:, ot:ot + 1], scale=1.0)
                    nc.sync.dma_start(
                        out=yT[base + ot * 128:base + (ot + 1) * 128, :], in_=yt[:])

    nc.finalize()
    return nc


def _prep_inputs(x, w_in, b_in, w_out, b_out):
    x = np.asarray(x, dtype=np.float32)
    w_in = np.asarray(w_in, dtype=np.float32)
    b_in = np.asarray(b_in, dtype=np.float32)
    w_out = np.asarray(w_out, dtype=np.float32)
    b_out = np.asarray(b_out, dtype=np.float32)

    # fold 1/sqrt(dh) into the q rows of w_in / b_in
    w_in_s = w_in.copy()
    b_in_s = b_in.copy()
    w_in_s[:C] *= DH ** -0.5
    b_in_s[:C] *= DH ** -0.5

    w_inT = np.ascontiguousarray(w_in_s.T).astype(BF16)
    w_outT = np.ascontiguousarray(w_out.T).astype(BF16)
    bqk = np.ascontiguousarray(b_in_s[:2 * C].reshape(16, 128).T, dtype=np.float32)
    # v bias folds exactly into an effective output bias:
    #   (p @ (v + 1 b_v^T)) / denom = (p @ v)/denom + b_v
    b_out_eff = b_out + w_out @ b_in[2 * C:]
    bout = np.ascontiguousarray(b_out_eff.reshape(8, 128).T, dtype=np.float32)

    i = np.arange(128)[:, None]
    j = np.arange(128)[None, :]
    tri = np.where(j <= i, np.float32(0), np.float32(-1e9))

    # dilated gather + transpose: (B, S, C, M) -> per-core (2C, M)
    xi = x.reshape(B, S, W_SEG, C)[:, :, ::RATE, :]        # (B, S, M, C)
    xiT = np.ascontiguousarray(xi.transpose(0, 1, 3, 2)).astype(BF16)  # (B,S,C,M)
    xiT = xiT.reshape(N_CORES, SEG_PER_CORE * C, M)

    in_maps = []
    for c in range(N_CORES):
        in_maps.append({
            "xiT": np.ascontiguousarray(xiT[c]),
            "w_inT": w_inT,
            "w_outT": w_outT,
            "bqk": bqk,
            "bout": bout,
            "tri": tri,
        })
    return in_maps


def kernel(x, w_in, b_in, w_out, b_out, _trace=False):
    if "nc" not in _CACHE:
        _CACHE["nc"] = _build()
    nc = _CACHE["nc"]

    in_maps = _prep_inputs(x, w_in, b_in, w_out, b_out)
    res = run_bass_kernel_spmd(
        nc, in_maps, core_ids=list(range(N_CORES)), trace=_trace)
    _CACHE["last_result"] = res

    out = np.zeros((B, N, C), dtype=np.float32)
    ov = out.reshape(B, S, W_SEG, C)
    for c in range(N_CORES):
        yTc = res.results[c]["yT"]                       # (2C, M) fp32
        for seg in range(SEG_PER_CORE):
            gseg = c * SEG_PER_CORE + seg
            b, s = divmod(gseg, S)
            ov[b, s, ::RATE, :] = yTc[seg * C:(seg + 1) * C, :].T
    return out


# revision 33
# speedup vs baseline: 1.6126x; 1.6126x over previous
# Dilated causal self-attention kernel for Trainium2 (8 NeuronCores).
#
# Reference computation (see problem):
#   x (4, 8192, 1024) -> reshape (4, 4, 2048, 1024) -> take every 4th token
#   -> per-segment causal MHA (16 heads, dh=64) -> scatter back into zeros.
#
# Sharding: 16 independent (batch, segment) attention problems, 2 per core.
# Host does the dilated gather + transpose + bf16 cast and the final scatter
# into the zero background; each core runs QKV -> per-head causal softmax
# attention -> output projection on its 2 segments.
#
# Device layout (all feature-major where possible):
#   xiT    [C, M]  (per segment)         - input, bf16
#   qkT    [2C, M] feature-major         - q rows pre-scaled by 1/sqrt(dh)
#                                          (folded into w_in on host)
#   v      [M, C]  token-major           - v bias folds into output bias
#   scores [128 q, n k] per (head, q-chunk), n = (qc+1)*128 (causal skip)
#   p = exp(scores) (no max subtraction: scores ~ N(0,1)), accum_out = denom
#   PV: outT[dh, M] = sum_kc v_kc^T @ pT_kc   (pT via PE transpose)
#   yT = w_outT^T @ oT + b_out_eff  -> DMA out feature-major

import sys

sys.path.insert(0, "/opt/trn_rl_repo")

import numpy as np
import ml_dtypes

import concourse.bacc as bacc
import concourse.mybir as mybir
from concourse.tile import TileContext
from concourse.bass_utils import run_bass_kernel_spmd
from concourse.masks import make_identity

BF16 = ml_dtypes.bfloat16

B, N, C = 4, 8192, 1024
W_SEG, RATE, H = 2048, 4, 16
DH = C // H            # 64
S = N // W_SEG         # 4 segments per batch
M = W_SEG // RATE      # 512 tokens per segment
N_CORES = 8
SEG_PER_CORE = (B * S) // N_CORES  # 2

FP32 = mybir.dt.float32
BF = mybir.dt.bfloat16

_CACHE = {}


def _build():
    nc = bacc.Bacc()
    phase_of = _CACHE.setdefault("phase_of", {})

    def mm(phase, *args, **kwargs):
        inst = nc.tensor.matmul(*args, **kwargs)
        try:
            phase_of[inst.ins.name] = phase
        except Exception:
            pass
        return inst
    # chunk-major packed layouts (one DMA each; see _prep_inputs)
    xiT = nc.dram_tensor("xiT", [SEG_PER_CORE * 128, 8 * M], BF, kind="ExternalInput")
    wqk = nc.dram_tensor("wqk", [4 * 128, 8 * 512], BF, kind="ExternalInput")
    wv = nc.dram_tensor("wv", [128, 8 * C], BF, kind="ExternalInput")
    wout = nc.dram_tensor("wout", [128, 8 * C], BF, kind="ExternalInput")
    bqk = nc.dram_tensor("bqk", [128, 16], FP32, kind="ExternalInput")
    bout = nc.dram_tensor("bout", [128, 8], FP32, kind="ExternalInput")
    tri = nc.dram_tensor("tri", [128, 128], FP32, kind="ExternalInput")
    yT = nc.dram_tensor("yT", [SEG_PER_CORE * C, M], FP32, kind="ExternalOutput")

    CT = C // 128  # 8 contraction chunks

    from contextlib import ExitStack
    with TileContext(nc) as tc, ExitStack() as ctx:
        consts = ctx.enter_context(tc.tile_pool(name="consts", bufs=1))
        wpool = ctx.enter_context(tc.tile_pool(name="weights", bufs=1))
        xpool = ctx.enter_context(tc.tile_pool(name="x", bufs=2))
        qkpool = ctx.enter_context(tc.tile_pool(name="qk", bufs=32))
        vpool = ctx.enter_context(tc.tile_pool(name="v", bufs=8))
        ptpool = ctx.enter_context(tc.tile_pool(name="pt", bufs=10))
        rbpool = ctx.enter_context(tc.tile_pool(name="rb", bufs=4))
        otpool = ctx.enter_context(tc.tile_pool(name="ot", bufs=8))
        ypool = ctx.enter_context(tc.tile_pool(name="y", bufs=3))
        small = ctx.enter_context(tc.tile_pool(name="small", bufs=4))
        psA = ctx.enter_context(tc.tile_pool(name="psA", bufs=2, space="PSUM"))
        psS = ctx.enter_context(tc.tile_pool(name="psS", bufs=4, space="PSUM"))
        psO = ctx.enter_context(tc.tile_pool(name="psO", bufs=2, space="PSUM"))

        if True:
            tri_sb = consts.tile([128, 128], FP32, tag="tri")
            nc.sync.dma_start(out=tri_sb[:], in_=tri[:, :])
            bqk_sb = consts.tile([128, 16], FP32, tag="bqk")
            nc.sync.dma_start(out=bqk_sb[:], in_=bqk[:, :])
            bout_sb = consts.tile([128, 8], FP32, tag="bout")
            nc.sync.dma_start(out=bout_sb[:], in_=bout[:, :])

            # wqk_sb[c4][:, ct*512 + off]: weights for qk pair 2*c4+pp,
            # ct-major within the chunk; wv/wout are [128, ct*1024 + col]
            wqk_sb = [wpool.tile([128, 8 * 512], BF, tag=f"wqk{c4}", name="w")
                      for c4 in range(4)]
            wv_sb = wpool.tile([128, 8 * C], BF, tag="wv")
            wout_sb = wpool.tile([128, 8 * C], BF, tag="wout")

            def emit_w_qk_chunk(c4):
                nc.sync.dma_start(
                    out=wqk_sb[c4][:], in_=wqk[c4 * 128:(c4 + 1) * 128, :])

            def emit_w_v():
                nc.sync.dma_start(out=wv_sb[:], in_=wv[:, :])

            def emit_w_out():
                nc.sync.dma_start(out=wout_sb[:], in_=wout[:, :])

            # --- software-pipelined emission ---------------------------------
            # Dense matmul phases (QKV, proj) are interleaved into the
            # attention phase so the PE never idles (HAM stays at 2.4 GHz):
            #   A(0) | B(0)+C(0) with A(1) spread through | B(1)+C(1)
            x_sb = {}
            qk_sb = {}
            v_sb = {}
            oT_sb = {}

            def emit_x(seg):
                t = xpool.tile([128, 8 * M], BF, tag="x", name="x")
                nc.sync.dma_start(
                    out=t[:], in_=xiT[seg * 128:(seg + 1) * 128, :])
                x_sb[seg] = t

            QK_ORDER = [p + half for p in range(8) for half in (0, 8)]

            def emit_qkv_unit(seg, u):
                # units 0..15: qk e-tiles (interleaved q/k); 16..23: v halves
                if u < 16:
                    et = QK_ORDER[u]
                    p = et % 8
                    c4, off = p // 2, (p % 2) * 256 + (0 if et < 8 else 128)
                    ps = psA.tile([128, M], FP32, tag="psA", name="ps")
                    for ct in range(CT):
                        mm("qkv_qk",
                            ps[:],
                            lhsT=wqk_sb[c4][:, ct * 512 + off:ct * 512 + off + 128],
                            rhs=x_sb[seg][:, ct * M:(ct + 1) * M],
                            start=(ct == 0), stop=(ct == CT - 1))
                    t = qkpool.tile([128, M], BF, tag="qk", name="qk")
                    nc.scalar.activation(
                        out=t[:], in_=ps[:],
                        func=mybir.ActivationFunctionType.Identity,
                        bias=bqk_sb[:, et:et + 1], scale=1.0)
                    qk_sb.setdefault(seg, [None] * 16)[et] = t
                else:
                    tt, nf = divmod(u - 16, 2)
                    if nf == 0:
                        vt = vpool.tile([128, 16, 65], BF, tag="v", name="v")
                        v_sb.setdefault(seg, [None] * 4)[tt] = vt
                        # ones column per head: PV row 64 accumulates the
                        # softmax denominator for free
                        nc.vector.memset(vt[:, :, 64:65], 1.0)
                    vt = v_sb[seg][tt]
                    ps = psA.tile([128, M], FP32, tag="psA", name="ps")
                    for ct in range(CT):
                        mm("qkv_v",
                            ps[:],
                            lhsT=x_sb[seg][:, ct * M + tt * 128:ct * M + (tt + 1) * 128],
                            rhs=wv_sb[:, ct * C + nf * 512:ct * C + (nf + 1) * 512],
                            start=(ct == 0), stop=(ct == CT - 1))
                    nc.scalar.copy(
                        out=vt[:, nf * 8:(nf + 1) * 8, 0:64],
                        in_=ps[:].rearrange("p (h e) -> p h e", e=64))

            def emit_scores(seg, h):
                # scoresT blocks [k, q]: lhsT = k-chunk, rhs = q (no
                # transposes needed anywhere; pT = exp(scoresT) directly)
                et, row = h // 2, (h % 2) * 64
                qh = qk_sb[seg][et][row:row + 64, :]
                kh = qk_sb[seg][8 + et][row:row + 64, :]
                pt_sb = []
                for kc in range(4):
                    n2 = (4 - kc) * 128
                    ps = psS.tile([128, M], FP32, tag="psS", name="ps")
                    mm("scores",
                        ps[:, :n2],
                        lhsT=kh[:, kc * 128:(kc + 1) * 128],
                        rhs=qh[:, kc * 128:], start=True, stop=True)
                    ptk = ptpool.tile([128, M], BF, tag="pt", name="pt")
                    nc.scalar.activation(
                        out=ptk[:, :n2], in_=ps[:, :n2],
                        func=mybir.ActivationFunctionType.Exp)
                    # causal mask: zero the lower triangle of the diagonal
                    # block (keep where q_local >= k_local) on idle GpSimd
                    nc.gpsimd.affine_select(
                        out=ptk[:, 0:128], in_=ptk[:, 0:128],
                        compare_op=mybir.AluOpType.is_ge,
                        fill=0.0, base=0,
                        pattern=[[1, 128]], channel_multiplier=-1)
                    pt_sb.append(ptk)
                return pt_sb

            def emit_pv(seg, h, po, pt_sb):
                # po [65, M]: rows 0:64 = unnormalized outT, row 64 = denom
                for kc in range(4):
                    n2 = (4 - kc) * 128
                    mm("pv",
                        po[:, kc * 128:],
                        lhsT=v_sb[seg][kc][:, h, :],
                        rhs=pt_sb[kc][:, :n2],
                        start=(kc == 0), stop=(kc == 3))
                denrow = small.tile([1, M], FP32, tag="denrow", name="denrow", bufs=3)
                nc.vector.tensor_copy(out=denrow[:], in_=po[64:65, :])
                rdenT = small.tile([1, M], FP32, tag="rdenT", name="rdenT", bufs=3)
                nc.vector.reciprocal_approx_fast(out=rdenT[:], in_=denrow[:])
                rb = rbpool.tile([64, M], FP32, tag="rb", name="rb")
                nc.gpsimd.partition_broadcast(rb[:], rdenT[:], channels=64)
                row = (h % 2) * 64
                nc.vector.tensor_mul(
                    out=oT_sb[seg][h // 2][row:row + 64, :],
                    in0=po[0:64, :], in1=rb[:])

            def emit_proj_tile(seg, ot):
                base = seg * C
                ps = psA.tile([128, M], FP32, tag="psA", name="ps")
                for ct in range(CT):
                    mm("proj",
                        ps[:],
                        lhsT=wout_sb[:, ct * C + ot * 128:ct * C + (ot + 1) * 128],
                        rhs=oT_sb[seg][ct][:],
                        start=(ct == 0), stop=(ct == CT - 1))
                yt = ypool.tile([128, M], FP32, tag="y", name="yt")
                nc.vector.tensor_scalar_add(yt[:], ps[:], bout_sb[:, ot:ot + 1])
                nc.sync.dma_start(
                    out=yT[base + ot * 128:base + (ot + 1) * 128, :], in_=yt[:])

            def emit_attn(seg, filler):
                # two-stage software pipeline over heads: scoresT+exp of head
                # h+1 is emitted before PV(h), covering softmax latency
                oT_sb[seg] = [otpool.tile([128, M], BF, tag="ot", name="ot")
                              for _ in range(8)]
                prev = None
                for h in range(H):
                    cur = (h, emit_scores(seg, h),
                           psO.tile([65, M], FP32, tag="psO", name="po"))
                    if prev is not None:
                        ph, pts, po = prev
                        emit_pv(seg, ph, po, pts)
                    filler()
                    prev = cur
                ph, pts, po = prev
                emit_pv(seg, ph, po, pts)

            emit_x(0)
            emit_w_qk_chunk(0)
            emit_w_qk_chunk(1)
            emit_w_v()
            emit_w_qk_chunk(2)
            emit_w_qk_chunk(3)
            emit_x(1)
            emit_w_out()
            # seg1 filler interleaves v into the qk stream (weights are
            # long since resident by then); seg0 keeps v last, since the wv
            # DMA lands after the qk weight chunks
            A_ORDER = [0, 1, 16, 2, 3, 17, 4, 5, 18, 6, 7, 19,
                       8, 9, 20, 10, 11, 21, 12, 13, 22, 14, 15, 23]
            for u in range(24):
                emit_qkv_unit(0, u)

            # B(0) with A(1) spread through; B(1) with C(0) spread through;
            # C(1) as the dense tail.
            qkv1 = iter(A_ORDER)

            def fill_qkv1():
                for _ in range(2):
                    u = next(qkv1, None)
                    if u is not None:
                        emit_qkv_unit(1, u)

            emit_attn(0, fill_qkv1)
            proj0 = iter(range(8))

            def fill_proj0():
                ot = next(proj0, None)
                if ot is not None:
                    emit_proj_tile(0, ot)

            emit_attn(1, fill_proj0)
            for ot in range(8):
                emit_proj_tile(1, ot)

    nc.finalize()
    return nc


def _prep_inputs(x, w_in, b_in, w_out, b_out):
    x = np.asarray(x, dtype=np.float32)
    w_in = np.asarray(w_in, dtype=np.float32)
    b_in = np.asarray(b_in, dtype=np.float32)
    w_out = np.asarray(w_out, dtype=np.float32)
    b_out = np.asarray(b_out, dtype=np.float32)

    # fold 1/sqrt(dh) into the q rows of w_in / b_in
    w_in_s = w_in.copy()
    b_in_s = b_in.copy()
    w_in_s[:C] *= DH ** -0.5
    b_in_s[:C] *= DH ** -0.5

    w_inT0 = np.ascontiguousarray(w_in_s.T).astype(BF16)
    # permute qk columns into [q_p | k_p] pairs matching the consume order
    w_inT = w_inT0.copy()
    for p in range(8):
        w_inT[:, p * 256:p * 256 + 128] = w_inT0[:, p * 128:(p + 1) * 128]
        w_inT[:, p * 256 + 128:(p + 1) * 256] = \
            w_inT0[:, C + p * 128:C + (p + 1) * 128]
    # repack into ct-major chunk layouts (one DMA per chunk on device)
    wp = w_inT.reshape(8, 128, 3 * C)
    wqk = np.ascontiguousarray(np.concatenate(
        [wp[:, :, c4 * 512:(c4 + 1) * 512].transpose(1, 0, 2).reshape(128, 8 * 512)
         for c4 in range(4)], axis=0))                       # (512, 4096)
    wv = np.ascontiguousarray(
        wp[:, :, 2 * C:].transpose(1, 0, 2).reshape(128, 8 * C))  # (128, 8192)
    w_outT = np.ascontiguousarray(w_out.T).astype(BF16)
    wout = np.ascontiguousarray(
        w_outT.reshape(8, 128, C).transpose(1, 0, 2).reshape(128, 8 * C))
    bqk = np.ascontiguousarray(b_in_s[:2 * C].reshape(16, 128).T, dtype=np.float32)
    # v bias folds exactly into an effective output bias:
    #   (p @ (v + 1 b_v^T)) / denom = (p @ v)/denom + b_v
    b_out_eff = b_out + w_out @ b_in[2 * C:]
    bout = np.ascontiguousarray(b_out_eff.reshape(8, 128).T, dtype=np.float32)

    # dilated gather + transpose + ct-major pack: per-core (2*128, 8*M)
    xi = x.reshape(B, S, W_SEG, C)[:, :, ::RATE, :]        # (B, S, M, C)
    xiT = np.ascontiguousarray(xi.transpose(0, 1, 3, 2)).astype(BF16)  # (B,S,C,M)
    xiT = xiT.reshape(16, 8, 128, M).transpose(0, 2, 1, 3)  # (16,128,8,M)
    xiT = np.ascontiguousarray(xiT).reshape(N_CORES, SEG_PER_CORE * 128, 8 * M)

    i = np.arange(128)[:, None]
    j = np.arange(128)[None, :]
    # scoresT orientation: rows = k, cols = q; valid iff q >= k
    tri = np.where(j >= i, np.float32(0), np.float32(-1e9))

    in_maps = []
    for c in range(N_CORES):
        in_maps.append({
            "xiT": np.ascontiguousarray(xiT[c]),
            "wqk": wqk,
            "wv": wv,
            "wout": wout,
            "bqk": bqk,
            "bout": bout,
            "tri": tri,
        })
    return in_maps


def kernel(x, w_in, b_in, w_out, b_out, _trace=False):
    if "nc" not in _CACHE:
        _CACHE["nc"] = _build()
    nc = _CACHE["nc"]

    in_maps = _prep_inputs(x, w_in, b_in, w_out, b_out)
    res = run_bass_kernel_spmd(
        nc, in_maps, core_ids=list(range(N_CORES)), trace=_trace)
    _CACHE["last_result"] = res

    out = np.zeros((B, N, C), dtype=np.float32)
    ov = out.reshape(B, S, W_SEG, C)
    for c in range(N_CORES):
        yTc = res.results[c]["yT"]                       # (2C, M) fp32
        for seg in range(SEG_PER_CORE):
            gseg = c * SEG_PER_CORE + seg
            b, s = divmod(gseg, S)
            ov[b, s, ::RATE, :] = yTc[seg * C:(seg + 1) * C, :].T
    return out


# revision 36
# speedup vs baseline: 1.6468x; 1.0213x over previous
# Dilated causal self-attention kernel for Trainium2 (8 NeuronCores).
#
# Reference computation (see problem):
#   x (4, 8192, 1024) -> reshape (4, 4, 2048, 1024) -> take every 4th token
#   -> per-segment causal MHA (16 heads, dh=64) -> scatter back into zeros.
#
# Sharding: 16 independent (batch, segment) attention problems, 2 per core.
# Host does the dilated gather + transpose + bf16 cast and the final scatter
# into the zero background; each core runs QKV -> per-head causal softmax
# attention -> output projection on its 2 segments.
#
# Device layout (all feature-major where possible):
#   xiT    [C, M]  (per segment)         - input, bf16
#   qkT    [2C, M] feature-major         - q rows pre-scaled by 1/sqrt(dh)
#                                          (folded into w_in on host)
#   v      [M, C]  token-major           - v bias folds into output bias
#   scores [128 q, n k] per (head, q-chunk), n = (qc+1)*128 (causal skip)
#   p = exp(scores) (no max subtraction: scores ~ N(0,1)), accum_out = denom
#   PV: outT[dh, M] = sum_kc v_kc^T @ pT_kc   (pT via PE transpose)
#   yT = w_outT^T @ oT + b_out_eff  -> DMA out feature-major

import sys

sys.path.insert(0, "/opt/trn_rl_repo")

import numpy as np
import ml_dtypes

import concourse.bacc as bacc
import concourse.mybir as mybir
from concourse.tile import TileContext
from concourse.bass_utils import run_bass_kernel_spmd
from concourse.masks import make_identity

BF16 = ml_dtypes.bfloat16

B, N, C = 4, 8192, 1024
W_SEG, RATE, H = 2048, 4, 16
DH = C // H            # 64
S = N // W_SEG         # 4 segments per batch
M = W_SEG // RATE      # 512 tokens per segment
N_CORES = 8
SEG_PER_CORE = (B * S) // N_CORES  # 2

FP32 = mybir.dt.float32
BF = mybir.dt.bfloat16

_CACHE = {}


def _build():
    nc = bacc.Bacc()
    phase_of = _CACHE.setdefault("phase_of", {})

    def mm(phase, *args, **kwargs):
        inst = nc.tensor.matmul(*args, **kwargs)
        try:
            phase_of[inst.ins.name] = phase
        except Exception:
            pass
        return inst
    # chunk-major packed layouts (one DMA each; see _prep_inputs)
    xiT = nc.dram_tensor("xiT", [SEG_PER_CORE * 128, 8 * M], BF, kind="ExternalInput")
    wqk = nc.dram_tensor("wqk", [4 * 128, 8 * 512], BF, kind="ExternalInput")
    wv = nc.dram_tensor("wv", [128, 8 * C], BF, kind="ExternalInput")
    wout = nc.dram_tensor("wout", [128, 8 * C], BF, kind="ExternalInput")
    bqk = nc.dram_tensor("bqk", [128, 16], FP32, kind="ExternalInput")
    bout = nc.dram_tensor("bout", [128, 8], FP32, kind="ExternalInput")
    tri = nc.dram_tensor("tri", [128, 128], FP32, kind="ExternalInput")
    yT = nc.dram_tensor("yT", [SEG_PER_CORE * C, M], FP32, kind="ExternalOutput")

    CT = C // 128  # 8 contraction chunks

    from contextlib import ExitStack
    with TileContext(nc) as tc, ExitStack() as ctx:
        consts = ctx.enter_context(tc.tile_pool(name="consts", bufs=1))
        wpool = ctx.enter_context(tc.tile_pool(name="weights", bufs=1))
        xpool = ctx.enter_context(tc.tile_pool(name="x", bufs=2))
        qkpool = ctx.enter_context(tc.tile_pool(name="qk", bufs=32))
        vpool = ctx.enter_context(tc.tile_pool(name="v", bufs=8))
        ptpool = ctx.enter_context(tc.tile_pool(name="pt", bufs=12))
        rbpool = ctx.enter_context(tc.tile_pool(name="rb", bufs=4))
        otpool = ctx.enter_context(tc.tile_pool(name="ot", bufs=8))
        ypool = ctx.enter_context(tc.tile_pool(name="y", bufs=3))
        small = ctx.enter_context(tc.tile_pool(name="small", bufs=4))
        psA = ctx.enter_context(tc.tile_pool(name="psA", bufs=2, space="PSUM"))
        psS = ctx.enter_context(tc.tile_pool(name="psS", bufs=4, space="PSUM"))
        psO = ctx.enter_context(tc.tile_pool(name="psO", bufs=2, space="PSUM"))

        if True:
            tri_sb = consts.tile([128, 128], FP32, tag="tri")
            nc.sync.dma_start(out=tri_sb[:], in_=tri[:, :])
            bqk_sb = consts.tile([128, 16], FP32, tag="bqk")
            nc.sync.dma_start(out=bqk_sb[:], in_=bqk[:, :])
            bout_sb = consts.tile([128, 8], FP32, tag="bout")
            nc.sync.dma_start(out=bout_sb[:], in_=bout[:, :])

            # wqk_sb[c4][:, ct*512 + off]: weights for qk pair 2*c4+pp,
            # ct-major within the chunk; wv/wout are [128, ct*1024 + col]
            wqk_sb = [wpool.tile([128, 8 * 512], BF, tag=f"wqk{c4}", name="w")
                      for c4 in range(4)]
            wv_sb = wpool.tile([128, 8 * C], BF, tag="wv")
            wout_sb = wpool.tile([128, 8 * C], BF, tag="wout")

            def emit_w_qk_chunk(c4):
                nc.sync.dma_start(
                    out=wqk_sb[c4][:], in_=wqk[c4 * 128:(c4 + 1) * 128, :])

            def emit_w_v():
                nc.sync.dma_start(out=wv_sb[:], in_=wv[:, :])

            def emit_w_out():
                nc.sync.dma_start(out=wout_sb[:], in_=wout[:, :])

            # --- software-pipelined emission ---------------------------------
            # Dense matmul phases (QKV, proj) are interleaved into the
            # attention phase so the PE never idles (HAM stays at 2.4 GHz):
            #   A(0) | B(0)+C(0) with A(1) spread through | B(1)+C(1)
            x_sb = {}
            qk_sb = {}
            v_sb = {}
            oT_sb = {}

            def emit_x(seg):
                t = xpool.tile([128, 8 * M], BF, tag="x", name="x")
                nc.sync.dma_start(
                    out=t[:], in_=xiT[seg * 128:(seg + 1) * 128, :])
                x_sb[seg] = t

            QK_ORDER = [p + half for p in range(8) for half in (0, 8)]

            def emit_qkv_unit(seg, u):
                # units 0..15: qk e-tiles (interleaved q/k); 16..23: v halves
                if u < 16:
                    et = QK_ORDER[u]
                    p = et % 8
                    c4, off = p // 2, (p % 2) * 256 + (0 if et < 8 else 128)
                    ps = psA.tile([128, M], FP32, tag="psA", name="ps")
                    for ct in range(CT):
                        mm("qkv_qk",
                            ps[:],
                            lhsT=wqk_sb[c4][:, ct * 512 + off:ct * 512 + off + 128],
                            rhs=x_sb[seg][:, ct * M:(ct + 1) * M],
                            start=(ct == 0), stop=(ct == CT - 1))
                    t = qkpool.tile([128, M], BF, tag="qk", name="qk")
                    nc.scalar.activation(
                        out=t[:], in_=ps[:],
                        func=mybir.ActivationFunctionType.Identity,
                        bias=bqk_sb[:, et:et + 1], scale=1.0)
                    qk_sb.setdefault(seg, [None] * 16)[et] = t
                else:
                    tt, nf = divmod(u - 16, 2)
                    if nf == 0:
                        vt = vpool.tile([128, 16, 65], BF, tag="v", name="v")
                        v_sb.setdefault(seg, [None] * 4)[tt] = vt
                        # ones column per head: PV row 64 accumulates the
                        # softmax denominator for free
                        nc.vector.memset(vt[:, :, 64:65], 1.0)
                    vt = v_sb[seg][tt]
                    ps = psA.tile([128, M], FP32, tag="psA", name="ps")
                    for ct in range(CT):
                        mm("qkv_v",
                            ps[:],
                            lhsT=x_sb[seg][:, ct * M + tt * 128:ct * M + (tt + 1) * 128],
                            rhs=wv_sb[:, ct * C + nf * 512:ct * C + (nf + 1) * 512],
                            start=(ct == 0), stop=(ct == CT - 1))
                    nc.scalar.copy(
                        out=vt[:, nf * 8:(nf + 1) * 8, 0:64],
                        in_=ps[:].rearrange("p (h e) -> p h e", e=64))

            def emit_scores(seg, h):
                # scoresT blocks [k, q]: lhsT = k-chunk, rhs = q (no
                # transposes needed anywhere; pT = exp(scoresT) directly)
                et, row = h // 2, (h % 2) * 64
                qh = qk_sb[seg][et][row:row + 64, :]
                kh = qk_sb[seg][8 + et][row:row + 64, :]
                pt_sb = []
                for kc in range(4):
                    n2 = (4 - kc) * 128
                    ps = psS.tile([128, M], FP32, tag="psS", name="ps")
                    mm("scores",
                        ps[:, :n2],
                        lhsT=kh[:, kc * 128:(kc + 1) * 128],
                        rhs=qh[:, kc * 128:], start=True, stop=True)
                    ptk = ptpool.tile([128, M], BF, tag="pt", name="pt")
                    nc.scalar.activation(
                        out=ptk[:, :n2], in_=ps[:, :n2],
                        func=mybir.ActivationFunctionType.Exp)
                    # causal mask: zero the lower triangle of the diagonal
                    # block (keep where q_local >= k_local) on idle GpSimd
                    nc.gpsimd.affine_select(
                        out=ptk[:, 0:128], in_=ptk[:, 0:128],
                        compare_op=mybir.AluOpType.is_ge,
                        fill=0.0, base=0,
                        pattern=[[1, 128]], channel_multiplier=-1)
                    pt_sb.append(ptk)
                return pt_sb

            def emit_pv(seg, h, po, pt_sb):
                # po [65, M]: rows 0:64 = unnormalized outT, row 64 = denom
                for kc in range(4):
                    n2 = (4 - kc) * 128
                    mm("pv",
                        po[:, kc * 128:],
                        lhsT=v_sb[seg][kc][:, h, :],
                        rhs=pt_sb[kc][:, :n2],
                        start=(kc == 0), stop=(kc == 3))
                denrow = small.tile([1, M], FP32, tag="denrow", name="denrow", bufs=3)
                nc.vector.tensor_copy(out=denrow[:], in_=po[64:65, :])
                rdenT = small.tile([1, M], FP32, tag="rdenT", name="rdenT", bufs=3)
                nc.vector.reciprocal_approx_fast(out=rdenT[:], in_=denrow[:])
                rb = rbpool.tile([64, M], FP32, tag="rb", name="rb")
                nc.gpsimd.partition_broadcast(rb[:], rdenT[:], channels=64)
                row = (h % 2) * 64
                nc.vector.tensor_mul(
                    out=oT_sb[seg][h // 2][row:row + 64, :],
                    in0=po[0:64, :], in1=rb[:])

            def emit_proj_tile(seg, ot):
                base = seg * C
                ps = psA.tile([128, M], FP32, tag="psA", name="ps")
                for ct in range(CT):
                    mm("proj",
                        ps[:],
                        lhsT=wout_sb[:, ct * C + ot * 128:ct * C + (ot + 1) * 128],
                        rhs=oT_sb[seg][ct][:],
                        start=(ct == 0), stop=(ct == CT - 1))
                yt = ypool.tile([128, M], FP32, tag="y", name="yt")
                nc.vector.tensor_scalar_add(yt[:], ps[:], bout_sb[:, ot:ot + 1])
                nc.sync.dma_start(
                    out=yT[base + ot * 128:base + (ot + 1) * 128, :], in_=yt[:])

            def emit_attn(seg, filler, warm=None):
                # two-stage software pipeline over heads: scoresT+exp of head
                # h+1 is emitted before PV(h), covering softmax latency.
                # `warm` carries heads whose scores were pre-emitted into the
                # preceding dense stream (pipeline warm-up).
                oT_sb[seg] = [otpool.tile([128, M], BF, tag="ot", name="ot")
                              for _ in range(8)]
                prev = None
                for h in range(H):
                    if warm and h in warm:
                        cur = (h,) + warm[h]
                    else:
                        cur = (h, emit_scores(seg, h),
                               psO.tile([65, M], FP32, tag="psO", name="po"))
                    if prev is not None:
                        ph, pts, po = prev
                        emit_pv(seg, ph, po, pts)
                    filler()
                    prev = cur
                ph, pts, po = prev
                emit_pv(seg, ph, po, pts)

            emit_x(0)
            emit_w_qk_chunk(0)
            emit_w_qk_chunk(1)
            emit_w_v()
            emit_w_qk_chunk(2)
            emit_w_qk_chunk(3)
            emit_x(1)
            emit_w_out()
            # seg1 filler interleaves v into the qk stream (weights are
            # long since resident by then); seg0 keeps v last, since the wv
            # DMA lands after the qk weight chunks
            A_ORDER = [0, 1, 16, 2, 3, 17, 4, 5, 18, 6, 7, 19,
                       8, 9, 20, 10, 11, 21, 12, 13, 22, 14, 15, 23]
            for u in range(20):
                emit_qkv_unit(0, u)
            warm0 = {0: (emit_scores(0, 0),
                         psO.tile([65, M], FP32, tag="psO", name="po"))}
            emit_qkv_unit(0, 20)
            emit_qkv_unit(0, 21)
            warm0[1] = (emit_scores(0, 1),
                        psO.tile([65, M], FP32, tag="psO", name="po"))
            emit_qkv_unit(0, 22)
            emit_qkv_unit(0, 23)

            # B(0) with A(1) spread through; B(1) with C(0) spread through;
            # C(1) as the dense tail.
            qkv1 = iter(A_ORDER)

            def fill_qkv1():
                for _ in range(2):
                    u = next(qkv1, None)
                    if u is not None:
                        emit_qkv_unit(1, u)

            emit_attn(0, fill_qkv1, warm=warm0)
            proj0 = iter(range(8))
            _pcall = [0]

            def fill_proj0():
                # emit on odd slots so the filler lasts the whole phase
                if _pcall[0] % 2 == 1:
                    ot = next(proj0, None)
                    if ot is not None:
                        emit_proj_tile(0, ot)
                _pcall[0] += 1

            emit_attn(1, fill_proj0)
            for ot in range(8):
                emit_proj_tile(1, ot)

    nc.finalize()
    return nc


def _prep_inputs(x, w_in, b_in, w_out, b_out):
    x = np.asarray(x, dtype=np.float32)
    w_in = np.asarray(w_in, dtype=np.float32)
    b_in = np.asarray(b_in, dtype=np.float32)
    w_out = np.asarray(w_out, dtype=np.float32)
    b_out = np.asarray(b_out, dtype=np.float32)

    # fold 1/sqrt(dh) into the q rows of w_in / b_in
    w_in_s = w_in.copy()
    b_in_s = b_in.copy()
    w_in_s[:C] *= DH ** -0.5
    b_in_s[:C] *= DH ** -0.5

    w_inT0 = np.ascontiguousarray(w_in_s.T).astype(BF16)
    # permute qk columns into [q_p | k_p] pairs matching the consume order
    w_inT = w_inT0.copy()
    for p in range(8):
        w_inT[:, p * 256:p * 256 + 128] = w_inT0[:, p * 128:(p + 1) * 128]
        w_inT[:, p * 256 + 128:(p + 1) * 256] = \
            w_inT0[:, C + p * 128:C + (p + 1) * 128]
    # repack into ct-major chunk layouts (one DMA per chunk on device)
    wp = w_inT.reshape(8, 128, 3 * C)
    wqk = np.ascontiguousarray(np.concatenate(
        [wp[:, :, c4 * 512:(c4 + 1) * 512].transpose(1, 0, 2).reshape(128, 8 * 512)
         for c4 in range(4)], axis=0))                       # (512, 4096)
    wv = np.ascontiguousarray(
        wp[:, :, 2 * C:].transpose(1, 0, 2).reshape(128, 8 * C))  # (128, 8192)
    w_outT = np.ascontiguousarray(w_out.T).astype(BF16)
    wout = np.ascontiguousarray(
        w_outT.reshape(8, 128, C).transpose(1, 0, 2).reshape(128, 8 * C))
    bqk = np.ascontiguousarray(b_in_s[:2 * C].reshape(16, 128).T, dtype=np.float32)
    # v bias folds exactly into an effective output bias:
    #   (p @ (v + 1 b_v^T)) / denom = (p @ v)/denom + b_v
    b_out_eff = b_out + w_out @ b_in[2 * C:]
    bout = np.ascontiguousarray(b_out_eff.reshape(8, 128).T, dtype=np.float32)

    # dilated gather + transpose + ct-major pack: per-core (2*128, 8*M)
    xi = x.reshape(B, S, W_SEG, C)[:, :, ::RATE, :]        # (B, S, M, C)
    xiT = np.ascontiguousarray(xi.transpose(0, 1, 3, 2)).astype(BF16)  # (B,S,C,M)
    xiT = xiT.reshape(16, 8, 128, M).transpose(0, 2, 1, 3)  # (16,128,8,M)
    xiT = np.ascontiguousarray(xiT).reshape(N_CORES, SEG_PER_CORE * 128, 8 * M)

    i = np.arange(128)[:, None]
    j = np.arange(128)[None, :]
    # scoresT orientation: rows = k, cols = q; valid iff q >= k
    tri = np.where(j >= i, np.float32(0), np.float32(-1e9))

    in_maps = []
    for c in range(N_CORES):
        in_maps.append({
            "xiT": np.ascontiguousarray(xiT[c]),
            "wqk": wqk,
            "wv": wv,
            "wout": wout,
            "bqk": bqk,
            "bout": bout,
            "tri": tri,
        })
    return in_maps


def kernel(x, w_in, b_in, w_out, b_out, _trace=False):
    if "nc" not in _CACHE:
        _CACHE["nc"] = _build()
    nc = _CACHE["nc"]

    in_maps = _prep_inputs(x, w_in, b_in, w_out, b_out)
    res = run_bass_kernel_spmd(
        nc, in_maps, core_ids=list(range(N_CORES)), trace=_trace)
    _CACHE["last_result"] = res

    out = np.zeros((B, N, C), dtype=np.float32)
    ov = out.reshape(B, S, W_SEG, C)
    for c in range(N_CORES):
        yTc = res.results[c]["yT"]                       # (2C, M) fp32
        for seg in range(SEG_PER_CORE):
            gseg = c * SEG_PER_CORE + seg
            b, s = divmod(gseg, S)
            ov[b, s, ::RATE, :] = yTc[seg * C:(seg + 1) * C, :].T
    return out


# revision 37
# speedup vs baseline: 1.6561x; 1.0056x over previous
# Dilated causal self-attention kernel for Trainium2 (8 NeuronCores).
#
# Reference computation (see problem):
#   x (4, 8192, 1024) -> reshape (4, 4, 2048, 1024) -> take every 4th token
#   -> per-segment causal MHA (16 heads, dh=64) -> scatter back into zeros.
#
# Sharding: 16 independent (batch, segment) attention problems, 2 per core.
# Host does the dilated gather + transpose + bf16 cast and the final scatter
# into the zero background; each core runs QKV -> per-head causal softmax
# attention -> output projection on its 2 segments.
#
# Device layout (all feature-major where possible):
#   xiT    [C, M]  (per segment)         - input, bf16
#   qkT    [2C, M] feature-major         - q rows pre-scaled by 1/sqrt(dh)
#                                          (folded into w_in on host)
#   v      [M, C]  token-major           - v bias folds into output bias
#   scores [128 q, n k] per (head, q-chunk), n = (qc+1)*128 (causal skip)
#   p = exp(scores) (no max subtraction: scores ~ N(0,1)), accum_out = denom
#   PV: outT[dh, M] = sum_kc v_kc^T @ pT_kc   (pT via PE transpose)
#   yT = w_outT^T @ oT + b_out_eff  -> DMA out feature-major

import sys

sys.path.insert(0, "/opt/trn_rl_repo")

import numpy as np
import ml_dtypes

import concourse.bacc as bacc
import concourse.mybir as mybir
from concourse.tile import TileContext
from concourse.bass_utils import run_bass_kernel_spmd
from concourse.masks import make_identity

BF16 = ml_dtypes.bfloat16

B, N, C = 4, 8192, 1024
W_SEG, RATE, H = 2048, 4, 16
DH = C // H            # 64
S = N // W_SEG         # 4 segments per batch
M = W_SEG // RATE      # 512 tokens per segment
N_CORES = 8
SEG_PER_CORE = (B * S) // N_CORES  # 2

FP32 = mybir.dt.float32
BF = mybir.dt.bfloat16

_CACHE = {}


def _build():
    nc = bacc.Bacc()
    phase_of = _CACHE.setdefault("phase_of", {})

    def mm(phase, *args, **kwargs):
        inst = nc.tensor.matmul(*args, **kwargs)
        try:
            phase_of[inst.ins.name] = phase
        except Exception:
            pass
        return inst
    # chunk-major packed layouts (one DMA each; see _prep_inputs)
    xiT = nc.dram_tensor("xiT", [SEG_PER_CORE * 128, 8 * M], BF, kind="ExternalInput")
    wqk = nc.dram_tensor("wqk", [4 * 128, 8 * 512], BF, kind="ExternalInput")
    wv = nc.dram_tensor("wv", [128, 8 * C], BF, kind="ExternalInput")
    wout = nc.dram_tensor("wout", [128, 8 * C], BF, kind="ExternalInput")
    bqk = nc.dram_tensor("bqk", [128, 16], FP32, kind="ExternalInput")
    bout = nc.dram_tensor("bout", [128, 8], FP32, kind="ExternalInput")
    tri = nc.dram_tensor("tri", [128, 128], FP32, kind="ExternalInput")
    yT = nc.dram_tensor("yT", [SEG_PER_CORE * C, M], FP32, kind="ExternalOutput")

    CT = C // 128  # 8 contraction chunks

    from contextlib import ExitStack
    with TileContext(nc) as tc, ExitStack() as ctx:
        consts = ctx.enter_context(tc.tile_pool(name="consts", bufs=1))
        wpool = ctx.enter_context(tc.tile_pool(name="weights", bufs=1))
        xpool = ctx.enter_context(tc.tile_pool(name="x", bufs=2))
        qkpool = ctx.enter_context(tc.tile_pool(name="qk", bufs=32))
        vpool = ctx.enter_context(tc.tile_pool(name="v", bufs=8))
        ptpool = ctx.enter_context(tc.tile_pool(name="pt", bufs=12))
        rbpool = ctx.enter_context(tc.tile_pool(name="rb", bufs=4))
        otpool = ctx.enter_context(tc.tile_pool(name="ot", bufs=8))
        ypool = ctx.enter_context(tc.tile_pool(name="y", bufs=3))
        small = ctx.enter_context(tc.tile_pool(name="small", bufs=4))
        psA = ctx.enter_context(tc.tile_pool(name="psA", bufs=2, space="PSUM"))
        psS = ctx.enter_context(tc.tile_pool(name="psS", bufs=4, space="PSUM"))
        psO = ctx.enter_context(tc.tile_pool(name="psO", bufs=2, space="PSUM"))

        if True:
            tri_sb = consts.tile([128, 128], FP32, tag="tri")
            bqk_sb = consts.tile([128, 16], FP32, tag="bqk")
            bout_sb = consts.tile([128, 8], FP32, tag="bout")

            # wqk_sb[c4][:, ct*512 + off]: weights for qk pair 2*c4+pp,
            # ct-major within the chunk; wv/wout are [128, ct*1024 + col]
            wqk_sb = [wpool.tile([128, 8 * 512], BF, tag=f"wqk{c4}", name="w")
                      for c4 in range(4)]
            wv_sb = wpool.tile([128, 8 * C], BF, tag="wv")
            wout_sb = wpool.tile([128, 8 * C], BF, tag="wout")

            def emit_w_qk_chunk(c4):
                if c4 == 0:
                    nc.sync.dma_start(
                        out=wqk_sb[0][:, :2048], in_=wqk[0:128, :2048])
                    nc.sync.dma_start(
                        out=wqk_sb[0][:, 2048:], in_=wqk[0:128, 2048:])
                else:
                    nc.sync.dma_start(
                        out=wqk_sb[c4][:], in_=wqk[c4 * 128:(c4 + 1) * 128, :])

            def emit_w_v():
                nc.sync.dma_start(out=wv_sb[:], in_=wv[:, :])

            def emit_w_out():
                nc.sync.dma_start(out=wout_sb[:], in_=wout[:, :])

            # --- software-pipelined emission ---------------------------------
            # Dense matmul phases (QKV, proj) are interleaved into the
            # attention phase so the PE never idles (HAM stays at 2.4 GHz):
            #   A(0) | B(0)+C(0) with A(1) spread through | B(1)+C(1)
            x_sb = {}
            qk_sb = {}
            v_sb = {}
            oT_sb = {}

            def emit_x(seg):
                t = xpool.tile([128, 8 * M], BF, tag="x", name="x")
                half = 4 * M
                nc.sync.dma_start(
                    out=t[:, :half], in_=xiT[seg * 128:(seg + 1) * 128, :half])
                nc.sync.dma_start(
                    out=t[:, half:], in_=xiT[seg * 128:(seg + 1) * 128, half:])
                x_sb[seg] = t

            QK_ORDER = [p + half for p in range(8) for half in (0, 8)]

            def emit_qkv_unit(seg, u):
                # units 0..15: qk e-tiles (interleaved q/k); 16..23: v halves
                if u < 16:
                    et = QK_ORDER[u]
                    p = et % 8
                    c4, off = p // 2, (p % 2) * 256 + (0 if et < 8 else 128)
                    ps = psA.tile([128, M], FP32, tag="psA", name="ps")
                    for ct in range(CT):
                        mm("qkv_qk",
                            ps[:],
                            lhsT=wqk_sb[c4][:, ct * 512 + off:ct * 512 + off + 128],
                            rhs=x_sb[seg][:, ct * M:(ct + 1) * M],
                            start=(ct == 0), stop=(ct == CT - 1))
                    t = qkpool.tile([128, M], BF, tag="qk", name="qk")
                    nc.scalar.activation(
                        out=t[:], in_=ps[:],
                        func=mybir.ActivationFunctionType.Identity,
                        bias=bqk_sb[:, et:et + 1], scale=1.0)
                    qk_sb.setdefault(seg, [None] * 16)[et] = t
                else:
                    tt, nf = divmod(u - 16, 2)
                    if nf == 0:
                        vt = vpool.tile([128, 16, 65], BF, tag="v", name="v")
                        v_sb.setdefault(seg, [None] * 4)[tt] = vt
                        # ones column per head: PV row 64 accumulates the
                        # softmax denominator for free
                        nc.vector.memset(vt[:, :, 64:65], 1.0)
                    vt = v_sb[seg][tt]
                    ps = psA.tile([128, M], FP32, tag="psA", name="ps")
                    for ct in range(CT):
                        mm("qkv_v",
                            ps[:],
                            lhsT=x_sb[seg][:, ct * M + tt * 128:ct * M + (tt + 1) * 128],
                            rhs=wv_sb[:, ct * C + nf * 512:ct * C + (nf + 1) * 512],
                            start=(ct == 0), stop=(ct == CT - 1))
                    nc.scalar.copy(
                        out=vt[:, nf * 8:(nf + 1) * 8, 0:64],
                        in_=ps[:].rearrange("p (h e) -> p h e", e=64))

            def emit_scores(seg, h):
                # scoresT blocks [k, q]: lhsT = k-chunk, rhs = q (no
                # transposes needed anywhere; pT = exp(scoresT) directly)
                et, row = h // 2, (h % 2) * 64
                qh = qk_sb[seg][et][row:row + 64, :]
                kh = qk_sb[seg][8 + et][row:row + 64, :]
                pt_sb = []
                for kc in range(4):
                    n2 = (4 - kc) * 128
                    ps = psS.tile([128, M], FP32, tag="psS", name="ps")
                    mm("scores",
                        ps[:, :n2],
                        lhsT=kh[:, kc * 128:(kc + 1) * 128],
                        rhs=qh[:, kc * 128:], start=True, stop=True)
                    ptk = ptpool.tile([128, M], BF, tag="pt", name="pt")
                    nc.scalar.activation(
                        out=ptk[:, :n2], in_=ps[:, :n2],
                        func=mybir.ActivationFunctionType.Exp)
                    # causal mask: zero the lower triangle of the diagonal
                    # block (keep where q_local >= k_local) on idle GpSimd
                    nc.gpsimd.affine_select(
                        out=ptk[:, 0:128], in_=ptk[:, 0:128],
                        compare_op=mybir.AluOpType.is_ge,
                        fill=0.0, base=0,
                        pattern=[[1, 128]], channel_multiplier=-1)
                    pt_sb.append(ptk)
                return pt_sb

            def emit_pv(seg, h, po, pt_sb):
                # po [65, M]: rows 0:64 = unnormalized outT, row 64 = denom
                for kc in range(4):
                    n2 = (4 - kc) * 128
                    mm("pv",
                        po[:, kc * 128:],
                        lhsT=v_sb[seg][kc][:, h, :],
                        rhs=pt_sb[kc][:, :n2],
                        start=(kc == 0), stop=(kc == 3))
                denrow = small.tile([1, M], FP32, tag="denrow", name="denrow", bufs=3)
                nc.vector.tensor_copy(out=denrow[:], in_=po[64:65, :])
                rdenT = small.tile([1, M], FP32, tag="rdenT", name="rdenT", bufs=3)
                nc.vector.reciprocal_approx_fast(out=rdenT[:], in_=denrow[:])
                rb = rbpool.tile([64, M], FP32, tag="rb", name="rb")
                nc.gpsimd.partition_broadcast(rb[:], rdenT[:], channels=64)
                row = (h % 2) * 64
                nc.vector.tensor_mul(
                    out=oT_sb[seg][h // 2][row:row + 64, :],
                    in0=po[0:64, :], in1=rb[:])

            def emit_proj_tile(seg, ot):
                base = seg * C
                ps = psA.tile([128, M], FP32, tag="psA", name="ps")
                for ct in range(CT):
                    mm("proj",
                        ps[:],
                        lhsT=wout_sb[:, ct * C + ot * 128:ct * C + (ot + 1) * 128],
                        rhs=oT_sb[seg][ct][:],
                        start=(ct == 0), stop=(ct == CT - 1))
                yt = ypool.tile([128, M], FP32, tag="y", name="yt")
                nc.vector.tensor_scalar_add(yt[:], ps[:], bout_sb[:, ot:ot + 1])
                nc.sync.dma_start(
                    out=yT[base + ot * 128:base + (ot + 1) * 128, :], in_=yt[:])

            def emit_attn(seg, filler, warm=None):
                # two-stage software pipeline over heads: scoresT+exp of head
                # h+1 is emitted before PV(h), covering softmax latency.
                # `warm` carries heads whose scores were pre-emitted into the
                # preceding dense stream (pipeline warm-up).
                oT_sb[seg] = [otpool.tile([128, M], BF, tag="ot", name="ot")
                              for _ in range(8)]
                prev = None
                for h in range(H):
                    if warm and h in warm:
                        cur = (h,) + warm[h]
                    else:
                        cur = (h, emit_scores(seg, h),
                               psO.tile([65, M], FP32, tag="psO", name="po"))
                    if prev is not None:
                        ph, pts, po = prev
                        emit_pv(seg, ph, po, pts)
                    filler()
                    prev = cur
                ph, pts, po = prev
                emit_pv(seg, ph, po, pts)

            emit_x(0)
            emit_w_qk_chunk(0)
            nc.sync.dma_start(out=tri_sb[:], in_=tri[:, :])
            nc.sync.dma_start(out=bqk_sb[:], in_=bqk[:, :])
            nc.sync.dma_start(out=bout_sb[:], in_=bout[:, :])
            emit_w_qk_chunk(1)
            emit_w_v()
            emit_w_qk_chunk(2)
            emit_w_qk_chunk(3)
            emit_x(1)
            emit_w_out()
            # seg1 filler interleaves v into the qk stream (weights are
            # long since resident by then); seg0 keeps v last, since the wv
            # DMA lands after the qk weight chunks
            A_ORDER = [0, 1, 16, 2, 3, 17, 4, 5, 18, 6, 7, 19,
                       8, 9, 20, 10, 11, 21, 12, 13, 22, 14, 15, 23]
            for u in range(20):
                emit_qkv_unit(0, u)
            warm0 = {0: (emit_scores(0, 0),
                         psO.tile([65, M], FP32, tag="psO", name="po"))}
            emit_qkv_unit(0, 20)
            emit_qkv_unit(0, 21)
            warm0[1] = (emit_scores(0, 1),
                        psO.tile([65, M], FP32, tag="psO", name="po"))
            emit_qkv_unit(0, 22)
            emit_qkv_unit(0, 23)

            # B(0) with A(1) spread through; B(1) with C(0) spread through;
            # C(1) as the dense tail.
            qkv1 = iter(A_ORDER)

            def fill_qkv1():
                for _ in range(2):
                    u = next(qkv1, None)
                    if u is not None:
                        emit_qkv_unit(1, u)

            emit_attn(0, fill_qkv1, warm=warm0)
            proj0 = iter(range(8))
            _pcall = [0]

            def fill_proj0():
                # emit on odd slots so the filler lasts the whole phase
                if _pcall[0] % 2 == 1:
                    ot = next(proj0, None)
                    if ot is not None:
                        emit_proj_tile(0, ot)
                _pcall[0] += 1

            emit_attn(1, fill_proj0)
            for ot in range(8):
                emit_proj_tile(1, ot)

    nc.finalize()
    return nc


def _prep_inputs(x, w_in, b_in, w_out, b_out):
    x = np.asarray(x, dtype=np.float32)
    w_in = np.asarray(w_in, dtype=np.float32)
    b_in = np.asarray(b_in, dtype=np.float32)
    w_out = np.asarray(w_out, dtype=np.float32)
    b_out = np.asarray(b_out, dtype=np.float32)

    # fold 1/sqrt(dh) into the q rows of w_in / b_in
    w_in_s = w_in.copy()
    b_in_s = b_in.copy()
    w_in_s[:C] *= DH ** -0.5
    b_in_s[:C] *= DH ** -0.5

    w_inT0 = np.ascontiguousarray(w_in_s.T).astype(BF16)
    # permute qk columns into [q_p | k_p] pairs matching the consume order
    w_inT = w_inT0.copy()
    for p in range(8):
        w_inT[:, p * 256:p * 256 + 128] = w_inT0[:, p * 128:(p + 1) * 128]
        w_inT[:, p * 256 + 128:(p + 1) * 256] = \
            w_inT0[:, C + p * 128:C + (p + 1) * 128]
    # repack into ct-major chunk layouts (one DMA per chunk on device)
    wp = w_inT.reshape(8, 128, 3 * C)
    wqk = np.ascontiguousarray(np.concatenate(
        [wp[:, :, c4 * 512:(c4 + 1) * 512].transpose(1, 0, 2).reshape(128, 8 * 512)
         for c4 in range(4)], axis=0))                       # (512, 4096)
    wv = np.ascontiguousarray(
        wp[:, :, 2 * C:].transpose(1, 0, 2).reshape(128, 8 * C))  # (128, 8192)
    w_outT = np.ascontiguousarray(w_out.T).astype(BF16)
    wout = np.ascontiguousarray(
        w_outT.reshape(8, 128, C).transpose(1, 0, 2).reshape(128, 8 * C))
    bqk = np.ascontiguousarray(b_in_s[:2 * C].reshape(16, 128).T, dtype=np.float32)
    # v bias folds exactly into an effective output bias:
    #   (p @ (v + 1 b_v^T)) / denom = (p @ v)/denom + b_v
    b_out_eff = b_out + w_out @ b_in[2 * C:]
    bout = np.ascontiguousarray(b_out_eff.reshape(8, 128).T, dtype=np.float32)

    # dilated gather + transpose + ct-major pack: per-core (2*128, 8*M)
    xi = x.reshape(B, S, W_SEG, C)[:, :, ::RATE, :]        # (B, S, M, C)
    xiT = np.ascontiguousarray(xi.transpose(0, 1, 3, 2)).astype(BF16)  # (B,S,C,M)
    xiT = xiT.reshape(16, 8, 128, M).transpose(0, 2, 1, 3)  # (16,128,8,M)
    xiT = np.ascontiguousarray(xiT).reshape(N_CORES, SEG_PER_CORE * 128, 8 * M)

    i = np.arange(128)[:, None]
    j = np.arange(128)[None, :]
    # scoresT orientation: rows = k, cols = q; valid iff q >= k
    tri = np.where(j >= i, np.float32(0), np.float32(-1e9))

    in_maps = []
    for c in range(N_CORES):
        in_maps.append({
            "xiT": np.ascontiguousarray(xiT[c]),
            "wqk": wqk,
            "wv": wv,
            "wout": wout,
            "bqk": bqk,
            "bout": bout,
            "tri": tri,
        })
    return in_maps


def kernel(x, w_in, b_in, w_out, b_out, _trace=False):
    if "nc" not in _CACHE:
        _CACHE["nc"] = _build()
    nc = _CACHE["nc"]

    in_maps = _prep_inputs(x, w_in, b_in, w_out, b_out)
    res = run_bass_kernel_spmd(
        nc, in_maps, core_ids=list(range(N_CORES)), trace=_trace)
    _CACHE["last_result"] = res

    out = np.zeros((B, N, C), dtype=np.float32)
    ov = out.reshape(B, S, W_SEG, C)
    for c in range(N_CORES):
        yTc = res.results[c]["yT"]                       # (2C, M) fp32
        for seg in range(SEG_PER_CORE):
            gseg = c * SEG_PER_CORE + seg
            b, s = divmod(gseg, S)
            ov[b, s, ::RATE, :] = yTc[seg * C:(seg + 1) * C, :].T
    return out


# revision 38
# speedup vs baseline: 1.6630x; 1.0042x over previous
# Dilated causal self-attention kernel for Trainium2 (8 NeuronCores).
#
# Reference computation (see problem):
#   x (4, 8192, 1024) -> reshape (4, 4, 2048, 1024) -> take every 4th token
#   -> per-segment causal MHA (16 heads, dh=64) -> scatter back into zeros.
#
# Sharding: 16 independent (batch, segment) attention problems, 2 per core.
# Host does the dilated gather + transpose + bf16 cast and the final scatter
# into the zero background; each core runs QKV -> per-head causal softmax
# attention -> output projection on its 2 segments.
#
# Device layout (all feature-major where possible):
#   xiT    [C, M]  (per segment)         - input, bf16
#   qkT    [2C, M] feature-major         - q rows pre-scaled by 1/sqrt(dh)
#                                          (folded into w_in on host)
#   v      [M, C]  token-major           - v bias folds into output bias
#   scores [128 q, n k] per (head, q-chunk), n = (qc+1)*128 (causal skip)
#   p = exp(scores) (no max subtraction: scores ~ N(0,1)), accum_out = denom
#   PV: outT[dh, M] = sum_kc v_kc^T @ pT_kc   (pT via PE transpose)
#   yT = w_outT^T @ oT + b_out_eff  -> DMA out feature-major

import sys

sys.path.insert(0, "/opt/trn_rl_repo")

import numpy as np
import ml_dtypes

import concourse.bacc as bacc
import concourse.mybir as mybir
from concourse.tile import TileContext
from concourse.bass_utils import run_bass_kernel_spmd
from concourse.masks import make_identity

BF16 = ml_dtypes.bfloat16

B, N, C = 4, 8192, 1024
W_SEG, RATE, H = 2048, 4, 16
DH = C // H            # 64
S = N // W_SEG         # 4 segments per batch
M = W_SEG // RATE      # 512 tokens per segment
N_CORES = 8
SEG_PER_CORE = (B * S) // N_CORES  # 2

FP32 = mybir.dt.float32
BF = mybir.dt.bfloat16

_CACHE = {}


def _build():
    nc = bacc.Bacc()
    phase_of = _CACHE.setdefault("phase_of", {})

    def mm(phase, *args, **kwargs):
        inst = nc.tensor.matmul(*args, **kwargs)
        try:
            phase_of[inst.ins.name] = phase
        except Exception:
            pass
        return inst
    # chunk-major packed layouts (one DMA each; see _prep_inputs)
    xiT = nc.dram_tensor("xiT", [SEG_PER_CORE * 128, 8 * M], BF, kind="ExternalInput")
    wqk = nc.dram_tensor("wqk", [4 * 128, 8 * 512], BF, kind="ExternalInput")
    wv = nc.dram_tensor("wv", [128, 8 * C], BF, kind="ExternalInput")
    wout = nc.dram_tensor("wout", [128, 8 * C], BF, kind="ExternalInput")
    bqk = nc.dram_tensor("bqk", [128, 16], FP32, kind="ExternalInput")
    bout = nc.dram_tensor("bout", [128, 8], FP32, kind="ExternalInput")
    tri = nc.dram_tensor("tri", [128, 128], FP32, kind="ExternalInput")
    yT = nc.dram_tensor("yT", [SEG_PER_CORE * C, M], FP32, kind="ExternalOutput")

    CT = C // 128  # 8 contraction chunks

    from contextlib import ExitStack
    with TileContext(nc) as tc, ExitStack() as ctx:
        consts = ctx.enter_context(tc.tile_pool(name="consts", bufs=1))
        wpool = ctx.enter_context(tc.tile_pool(name="weights", bufs=1))
        xpool = ctx.enter_context(tc.tile_pool(name="x", bufs=2))
        qkpool = ctx.enter_context(tc.tile_pool(name="qk", bufs=32))
        vpool = ctx.enter_context(tc.tile_pool(name="v", bufs=8))
        ptpool = ctx.enter_context(tc.tile_pool(name="pt", bufs=12))
        rbpool = ctx.enter_context(tc.tile_pool(name="rb", bufs=4))
        otpool = ctx.enter_context(tc.tile_pool(name="ot", bufs=8))
        ypool = ctx.enter_context(tc.tile_pool(name="y", bufs=3))
        small = ctx.enter_context(tc.tile_pool(name="small", bufs=4))
        psA = ctx.enter_context(tc.tile_pool(name="psA", bufs=2, space="PSUM"))
        psS = ctx.enter_context(tc.tile_pool(name="psS", bufs=4, space="PSUM"))
        psO = ctx.enter_context(tc.tile_pool(name="psO", bufs=2, space="PSUM"))

        if True:
            tri_sb = consts.tile([128, 128], FP32, tag="tri")
            bqk_sb = consts.tile([128, 16], FP32, tag="bqk")
            bout_sb = consts.tile([128, 8], FP32, tag="bout")

            # wqk_sb[c4][:, ct*512 + off]: weights for qk pair 2*c4+pp,
            # ct-major within the chunk; wv/wout are [128, ct*1024 + col]
            wqk_sb = [wpool.tile([128, 8 * 512], BF, tag=f"wqk{c4}", name="w")
                      for c4 in range(4)]
            wv_sb = wpool.tile([128, 8 * C], BF, tag="wv")
            wout_sb = wpool.tile([128, 8 * C], BF, tag="wout")

            def emit_w_qk_chunk(c4):
                if c4 == 0:
                    nc.sync.dma_start(
                        out=wqk_sb[0][:, :2048], in_=wqk[0:128, :2048])
                    nc.sync.dma_start(
                        out=wqk_sb[0][:, 2048:], in_=wqk[0:128, 2048:])
                else:
                    nc.sync.dma_start(
                        out=wqk_sb[c4][:], in_=wqk[c4 * 128:(c4 + 1) * 128, :])

            def emit_w_v():
                nc.sync.dma_start(out=wv_sb[:], in_=wv[:, :])

            def emit_w_out():
                nc.sync.dma_start(out=wout_sb[:], in_=wout[:, :])

            # --- software-pipelined emission ---------------------------------
            # Dense matmul phases (QKV, proj) are interleaved into the
            # attention phase so the PE never idles (HAM stays at 2.4 GHz):
            #   A(0) | B(0)+C(0) with A(1) spread through | B(1)+C(1)
            x_sb = {}
            qk_sb = {}
            v_sb = {}
            oT_sb = {}

            def emit_x(seg):
                t = xpool.tile([128, 8 * M], BF, tag="x", name="x")
                half = 4 * M
                nc.sync.dma_start(
                    out=t[:, :half], in_=xiT[seg * 128:(seg + 1) * 128, :half])
                nc.sync.dma_start(
                    out=t[:, half:], in_=xiT[seg * 128:(seg + 1) * 128, half:])
                x_sb[seg] = t

            QK_ORDER = [p + half for p in range(8) for half in (0, 8)]

            def emit_qkv_unit(seg, u):
                # units 0..15: qk e-tiles (interleaved q/k); 16..23: v halves
                if u < 16:
                    et = QK_ORDER[u]
                    p = et % 8
                    c4, off = p // 2, (p % 2) * 256 + (0 if et < 8 else 128)
                    ps = psA.tile([128, M], FP32, tag="psA", name="ps")
                    for ct in range(CT):
                        mm("qkv_qk",
                            ps[:],
                            lhsT=wqk_sb[c4][:, ct * 512 + off:ct * 512 + off + 128],
                            rhs=x_sb[seg][:, ct * M:(ct + 1) * M],
                            start=(ct == 0), stop=(ct == CT - 1))
                    t = qkpool.tile([128, M], BF, tag="qk", name="qk")
                    nc.scalar.activation(
                        out=t[:], in_=ps[:],
                        func=mybir.ActivationFunctionType.Identity,
                        bias=bqk_sb[:, et:et + 1], scale=1.0)
                    qk_sb.setdefault(seg, [None] * 16)[et] = t
                else:
                    tt, nf = divmod(u - 16, 2)
                    if nf == 0:
                        vt = vpool.tile([128, 16, 65], BF, tag="v", name="v")
                        v_sb.setdefault(seg, [None] * 4)[tt] = vt
                        # ones column per head: PV row 64 accumulates the
                        # softmax denominator for free
                        nc.vector.memset(vt[:, :, 64:65], 1.0)
                    vt = v_sb[seg][tt]
                    ps = psA.tile([128, M], FP32, tag="psA", name="ps")
                    for ct in range(CT):
                        mm("qkv_v",
                            ps[:],
                            lhsT=x_sb[seg][:, ct * M + tt * 128:ct * M + (tt + 1) * 128],
                            rhs=wv_sb[:, ct * C + nf * 512:ct * C + (nf + 1) * 512],
                            start=(ct == 0), stop=(ct == CT - 1))
                    nc.scalar.copy(
                        out=vt[:, nf * 8:(nf + 1) * 8, 0:64],
                        in_=ps[:].rearrange("p (h e) -> p h e", e=64))

            def emit_scores(seg, h):
                # scoresT blocks [k, q]: lhsT = k-chunk, rhs = q (no
                # transposes needed anywhere; pT = exp(scoresT) directly)
                et, row = h // 2, (h % 2) * 64
                qh = qk_sb[seg][et][row:row + 64, :]
                kh = qk_sb[seg][8 + et][row:row + 64, :]
                pt_sb = []
                for kc in range(4):
                    n2 = (4 - kc) * 128
                    ps = psS.tile([128, M], FP32, tag="psS", name="ps")
                    mm("scores",
                        ps[:, :n2],
                        lhsT=kh[:, kc * 128:(kc + 1) * 128],
                        rhs=qh[:, kc * 128:], start=True, stop=True)
                    ptk = ptpool.tile([128, M], BF, tag="pt", name="pt")
                    nc.scalar.activation(
                        out=ptk[:, :n2], in_=ps[:, :n2],
                        func=mybir.ActivationFunctionType.Exp)
                    # causal mask: zero the lower triangle of the diagonal
                    # block (keep where q_local >= k_local) on idle GpSimd
                    nc.gpsimd.affine_select(
                        out=ptk[:, 0:128], in_=ptk[:, 0:128],
                        compare_op=mybir.AluOpType.is_ge,
                        fill=0.0, base=0,
                        pattern=[[1, 128]], channel_multiplier=-1)
                    pt_sb.append(ptk)
                return pt_sb

            def emit_pv(seg, h, po, pt_sb):
                # po [65, M]: rows 0:64 = unnormalized outT, row 64 = denom
                for kc in range(4):
                    n2 = (4 - kc) * 128
                    mm("pv",
                        po[:, kc * 128:],
                        lhsT=v_sb[seg][kc][:, h, :],
                        rhs=pt_sb[kc][:, :n2],
                        start=(kc == 0), stop=(kc == 3))
                denrow = small.tile([1, M], FP32, tag="denrow", name="denrow", bufs=3)
                nc.vector.tensor_copy(out=denrow[:], in_=po[64:65, :])
                rdenT = small.tile([1, M], FP32, tag="rdenT", name="rdenT", bufs=3)
                nc.vector.reciprocal_approx_fast(out=rdenT[:], in_=denrow[:])
                rb = rbpool.tile([64, M], FP32, tag="rb", name="rb")
                nc.gpsimd.partition_broadcast(rb[:], rdenT[:], channels=64)
                row = (h % 2) * 64
                nc.vector.tensor_mul(
                    out=oT_sb[seg][h // 2][row:row + 64, :],
                    in0=po[0:64, :], in1=rb[:])

            def emit_proj_tile(seg, ot):
                base = seg * C
                ps = psA.tile([128, M], FP32, tag="psA", name="ps")
                for ct in range(CT):
                    mm("proj",
                        ps[:],
                        lhsT=wout_sb[:, ct * C + ot * 128:ct * C + (ot + 1) * 128],
                        rhs=oT_sb[seg][ct][:],
                        start=(ct == 0), stop=(ct == CT - 1))
                yt = ypool.tile([128, M], FP32, tag="y", name="yt")
                nc.vector.tensor_scalar_add(yt[:], ps[:], bout_sb[:, ot:ot + 1])
                nc.sync.dma_start(
                    out=yT[base + ot * 128:base + (ot + 1) * 128, :], in_=yt[:])

            def emit_attn(seg, filler, warm=None):
                # two-stage software pipeline over heads: scoresT+exp of head
                # h+1 is emitted before PV(h), covering softmax latency.
                # `warm` carries heads whose scores were pre-emitted into the
                # preceding dense stream (pipeline warm-up).
                oT_sb[seg] = [otpool.tile([128, M], BF, tag="ot", name="ot")
                              for _ in range(8)]
                prev = None
                for h in range(H):
                    if warm and h in warm:
                        cur = (h,) + warm[h]
                    else:
                        cur = (h, emit_scores(seg, h),
                               psO.tile([65, M], FP32, tag="psO", name="po"))
                    if prev is not None:
                        ph, pts, po = prev
                        emit_pv(seg, ph, po, pts)
                    filler()
                    prev = cur
                ph, pts, po = prev
                emit_pv(seg, ph, po, pts)

            emit_x(0)
            emit_w_qk_chunk(0)
            nc.sync.dma_start(out=tri_sb[:], in_=tri[:, :])
            nc.sync.dma_start(out=bqk_sb[:], in_=bqk[:, :])
            nc.sync.dma_start(out=bout_sb[:], in_=bout[:, :])
            emit_w_qk_chunk(1)
            emit_w_v()
            emit_w_qk_chunk(2)
            emit_w_qk_chunk(3)
            emit_x(1)
            emit_w_out()
            # seg1 filler interleaves v into the qk stream (weights are
            # long since resident by then); seg0 keeps v last, since the wv
            # DMA lands after the qk weight chunks
            A_ORDER = [0, 1, 16, 2, 3, 17, 4, 5, 18, 6, 7, 19,
                       8, 9, 20, 10, 11, 21, 12, 13, 22, 14, 15, 23]
            for u in range(20):
                emit_qkv_unit(0, u)
            warm0 = {0: (emit_scores(0, 0),
                         psO.tile([65, M], FP32, tag="psO", name="po"))}
            emit_qkv_unit(0, 20)
            emit_qkv_unit(0, 21)
            warm0[1] = (emit_scores(0, 1),
                        psO.tile([65, M], FP32, tag="psO", name="po"))
            emit_qkv_unit(0, 22)
            emit_qkv_unit(0, 23)

            # B(0) with A(1) spread through; B(1) with C(0) spread through;
            # C(1) as the dense tail.
            qkv1 = iter(A_ORDER)

            def fill_qkv1():
                for _ in range(2):
                    u = next(qkv1, None)
                    if u is not None:
                        emit_qkv_unit(1, u)

            emit_attn(0, fill_qkv1, warm=warm0)
            # warm-start seg1's pipeline the same way: its first two heads'
            # score chains begin while seg0's tail PV work runs on the PE
            warm1 = {0: (emit_scores(1, 0),
                         psO.tile([65, M], FP32, tag="psO", name="po")),
                     1: (emit_scores(1, 1),
                         psO.tile([65, M], FP32, tag="psO", name="po"))}
            proj0 = iter(range(8))
            _pcall = [0]

            def fill_proj0():
                # emit on odd slots so the filler lasts the whole phase
                if _pcall[0] % 2 == 1:
                    ot = next(proj0, None)
                    if ot is not None:
                        emit_proj_tile(0, ot)
                _pcall[0] += 1

            emit_attn(1, fill_proj0, warm=warm1)
            for ot in range(8):
                emit_proj_tile(1, ot)

    nc.finalize()
    return nc


def _prep_inputs(x, w_in, b_in, w_out, b_out):
    x = np.asarray(x, dtype=np.float32)
    w_in = np.asarray(w_in, dtype=np.float32)
    b_in = np.asarray(b_in, dtype=np.float32)
    w_out = np.asarray(w_out, dtype=np.float32)
    b_out = np.asarray(b_out, dtype=np.float32)

    # fold 1/sqrt(dh) into the q rows of w_in / b_in
    w_in_s = w_in.copy()
    b_in_s = b_in.copy()
    w_in_s[:C] *= DH ** -0.5
    b_in_s[:C] *= DH ** -0.5

    w_inT0 = np.ascontiguousarray(w_in_s.T).astype(BF16)
    # permute qk columns into [q_p | k_p] pairs matching the consume order
    w_inT = w_inT0.copy()
    for p in range(8):
        w_inT[:, p * 256:p * 256 + 128] = w_inT0[:, p * 128:(p + 1) * 128]
        w_inT[:, p * 256 + 128:(p + 1) * 256] = \
            w_inT0[:, C + p * 128:C + (p + 1) * 128]
    # repack into ct-major chunk layouts (one DMA per chunk on device)
    wp = w_inT.reshape(8, 128, 3 * C)
    wqk = np.ascontiguousarray(np.concatenate(
        [wp[:, :, c4 * 512:(c4 + 1) * 512].transpose(1, 0, 2).reshape(128, 8 * 512)
         for c4 in range(4)], axis=0))                       # (512, 4096)
    wv = np.ascontiguousarray(
        wp[:, :, 2 * C:].transpose(1, 0, 2).reshape(128, 8 * C))  # (128, 8192)
    w_outT = np.ascontiguousarray(w_out.T).astype(BF16)
    wout = np.ascontiguousarray(
        w_outT.reshape(8, 128, C).transpose(1, 0, 2).reshape(128, 8 * C))
    bqk = np.ascontiguousarray(b_in_s[:2 * C].reshape(16, 128).T, dtype=np.float32)
    # v bias folds exactly into an effective output bias:
    #   (p @ (v + 1 b_v^T)) / denom = (p @ v)/denom + b_v
    b_out_eff = b_out + w_out @ b_in[2 * C:]
    bout = np.ascontiguousarray(b_out_eff.reshape(8, 128).T, dtype=np.float32)

    # dilated gather + transpose + ct-major pack: per-core (2*128, 8*M)
    xi = x.reshape(B, S, W_SEG, C)[:, :, ::RATE, :]        # (B, S, M, C)
    xiT = np.ascontiguousarray(xi.transpose(0, 1, 3, 2)).astype(BF16)  # (B,S,C,M)
    xiT = xiT.reshape(16, 8, 128, M).transpose(0, 2, 1, 3)  # (16,128,8,M)
    xiT = np.ascontiguousarray(xiT).reshape(N_CORES, SEG_PER_CORE * 128, 8 * M)

    i = np.arange(128)[:, None]
    j = np.arange(128)[None, :]
    # scoresT orientation: rows = k, cols = q; valid iff q >= k
    tri = np.where(j >= i, np.float32(0), np.float32(-1e9))

    in_maps = []
    for c in range(N_CORES):
        in_maps.append({
            "xiT": np.ascontiguousarray(xiT[c]),
            "wqk": wqk,
            "wv": wv,
            "wout": wout,
            "bqk": bqk,
            "bout": bout,
            "tri": tri,
        })
    return in_maps


def kernel(x, w_in, b_in, w_out, b_out, _trace=False):
    if "nc" not in _CACHE:
        _CACHE["nc"] = _build()
    nc = _CACHE["nc"]

    in_maps = _prep_inputs(x, w_in, b_in, w_out, b_out)
    res = run_bass_kernel_spmd(
        nc, in_maps, core_ids=list(range(N_CORES)), trace=_trace)
    _CACHE["last_result"] = res

    out = np.zeros((B, N, C), dtype=np.float32)
    ov = out.reshape(B, S, W_SEG, C)
    for c in range(N_CORES):
        yTc = res.results[c]["yT"]                       # (2C, M) fp32
        for seg in range(SEG_PER_CORE):
            gseg = c * SEG_PER_CORE + seg
            b, s = divmod(gseg, S)
            ov[b, s, ::RATE, :] = yTc[seg * C:(seg + 1) * C, :].T
    return out


# revision 39
# speedup vs baseline: 1.6709x; 1.0047x over previous
# Dilated causal self-attention kernel for Trainium2 (8 NeuronCores).
#
# Reference computation (see problem):
#   x (4, 8192, 1024) -> reshape (4, 4, 2048, 1024) -> take every 4th token
#   -> per-segment causal MHA (16 heads, dh=64) -> scatter back into zeros.
#
# Sharding: 16 independent (batch, segment) attention problems, 2 per core.
# Host does the dilated gather + transpose + bf16 cast and the final scatter
# into the zero background; each core runs QKV -> per-head causal softmax
# attention -> output projection on its 2 segments.
#
# Device layout (all feature-major where possible):
#   xiT    [C, M]  (per segment)         - input, bf16
#   qkT    [2C, M] feature-major         - q rows pre-scaled by 1/sqrt(dh)
#                                          (folded into w_in on host)
#   v      [M, C]  token-major           - v bias folds into output bias
#   scores [128 q, n k] per (head, q-chunk), n = (qc+1)*128 (causal skip)
#   p = exp(scores) (no max subtraction: scores ~ N(0,1)), accum_out = denom
#   PV: outT[dh, M] = sum_kc v_kc^T @ pT_kc   (pT via PE transpose)
#   yT = w_outT^T @ oT + b_out_eff  -> DMA out feature-major

import sys

sys.path.insert(0, "/opt/trn_rl_repo")

import numpy as np
import ml_dtypes

import concourse.bacc as bacc
import concourse.mybir as mybir
from concourse.tile import TileContext
from concourse.bass_utils import run_bass_kernel_spmd
from concourse.masks import make_identity

BF16 = ml_dtypes.bfloat16

B, N, C = 4, 8192, 1024
W_SEG, RATE, H = 2048, 4, 16
DH = C // H            # 64
S = N // W_SEG         # 4 segments per batch
M = W_SEG // RATE      # 512 tokens per segment
N_CORES = 8
SEG_PER_CORE = (B * S) // N_CORES  # 2

FP32 = mybir.dt.float32
BF = mybir.dt.bfloat16

_CACHE = {}


def _build():
    nc = bacc.Bacc()
    phase_of = _CACHE.setdefault("phase_of", {})

    def mm(phase, *args, **kwargs):
        inst = nc.tensor.matmul(*args, **kwargs)
        try:
            phase_of[inst.ins.name] = phase
        except Exception:
            pass
        return inst
    # chunk-major packed layouts (one DMA each; see _prep_inputs)
    xiT = nc.dram_tensor("xiT", [SEG_PER_CORE * 128, 8 * M], BF, kind="ExternalInput")
    wqk = nc.dram_tensor("wqk", [4 * 128, 8 * 512], BF, kind="ExternalInput")
    wv = nc.dram_tensor("wv", [128, 8 * C], BF, kind="ExternalInput")
    wout = nc.dram_tensor("wout", [128, 8 * C], BF, kind="ExternalInput")
    bqk = nc.dram_tensor("bqk", [128, 16], FP32, kind="ExternalInput")
    bout = nc.dram_tensor("bout", [128, 8], FP32, kind="ExternalInput")
    tri = nc.dram_tensor("tri", [128, 128], FP32, kind="ExternalInput")
    yT = nc.dram_tensor("yT", [SEG_PER_CORE * C, M], FP32, kind="ExternalOutput")

    CT = C // 128  # 8 contraction chunks

    from contextlib import ExitStack
    with TileContext(nc) as tc, ExitStack() as ctx:
        consts = ctx.enter_context(tc.tile_pool(name="consts", bufs=1))
        wpool = ctx.enter_context(tc.tile_pool(name="weights", bufs=1))
        xpool = ctx.enter_context(tc.tile_pool(name="x", bufs=2))
        qkpool = ctx.enter_context(tc.tile_pool(name="qk", bufs=32))
        vpool = ctx.enter_context(tc.tile_pool(name="v", bufs=8))
        ptpool = ctx.enter_context(tc.tile_pool(name="pt", bufs=12))
        rbpool = ctx.enter_context(tc.tile_pool(name="rb", bufs=4))
        otpool = ctx.enter_context(tc.tile_pool(name="ot", bufs=8))
        ypool = ctx.enter_context(tc.tile_pool(name="y", bufs=3))
        small = ctx.enter_context(tc.tile_pool(name="small", bufs=4))
        psA = ctx.enter_context(tc.tile_pool(name="psA", bufs=2, space="PSUM"))
        psS = ctx.enter_context(tc.tile_pool(name="psS", bufs=4, space="PSUM"))
        psO = ctx.enter_context(tc.tile_pool(name="psO", bufs=2, space="PSUM"))

        if True:
            tri_sb = consts.tile([128, 128], FP32, tag="tri")
            bqk_sb = consts.tile([128, 16], FP32, tag="bqk")
            bout_sb = consts.tile([128, 8], FP32, tag="bout")

            # wqk_sb[c4][:, ct*512 + off]: weights for qk pair 2*c4+pp,
            # ct-major within the chunk; wv/wout are [128, ct*1024 + col]
            wqk_sb = [wpool.tile([128, 8 * 512], BF, tag=f"wqk{c4}", name="w")
                      for c4 in range(4)]
            wv_sb = wpool.tile([128, 8 * C], BF, tag="wv")
            wout_sb = wpool.tile([128, 8 * C], BF, tag="wout")

            def emit_w_qk_chunk(c4):
                if c4 == 0:
                    for i in range(4):
                        nc.sync.dma_start(
                            out=wqk_sb[0][:, i * 1024:(i + 1) * 1024],
                            in_=wqk[0:128, i * 1024:(i + 1) * 1024])
                else:
                    nc.sync.dma_start(
                        out=wqk_sb[c4][:], in_=wqk[c4 * 128:(c4 + 1) * 128, :])

            def emit_w_v():
                nc.sync.dma_start(out=wv_sb[:], in_=wv[:, :])

            def emit_w_out():
                nc.sync.dma_start(out=wout_sb[:], in_=wout[:, :])

            # --- software-pipelined emission ---------------------------------
            # Dense matmul phases (QKV, proj) are interleaved into the
            # attention phase so the PE never idles (HAM stays at 2.4 GHz):
            #   A(0) | B(0)+C(0) with A(1) spread through | B(1)+C(1)
            x_sb = {}
            qk_sb = {}
            v_sb = {}
            oT_sb = {}

            def emit_x(seg):
                t = xpool.tile([128, 8 * M], BF, tag="x", name="x")
                nq = 4 if seg == 0 else 2
                step = 8 * M // nq
                for i in range(nq):
                    nc.sync.dma_start(
                        out=t[:, i * step:(i + 1) * step],
                        in_=xiT[seg * 128:(seg + 1) * 128, i * step:(i + 1) * step])
                x_sb[seg] = t

            QK_ORDER = [p + half for p in range(8) for half in (0, 8)]

            def emit_qkv_unit(seg, u):
                # units 0..15: qk e-tiles (interleaved q/k); 16..23: v halves
                if u < 16:
                    et = QK_ORDER[u]
                    p = et % 8
                    c4, off = p // 2, (p % 2) * 256 + (0 if et < 8 else 128)
                    ps = psA.tile([128, M], FP32, tag="psA", name="ps")
                    for ct in range(CT):
                        mm("qkv_qk",
                            ps[:],
                            lhsT=wqk_sb[c4][:, ct * 512 + off:ct * 512 + off + 128],
                            rhs=x_sb[seg][:, ct * M:(ct + 1) * M],
                            start=(ct == 0), stop=(ct == CT - 1))
                    t = qkpool.tile([128, M], BF, tag="qk", name="qk")
                    nc.scalar.activation(
                        out=t[:], in_=ps[:],
                        func=mybir.ActivationFunctionType.Identity,
                        bias=bqk_sb[:, et:et + 1], scale=1.0)
                    qk_sb.setdefault(seg, [None] * 16)[et] = t
                else:
                    tt, nf = divmod(u - 16, 2)
                    if nf == 0:
                        vt = vpool.tile([128, 16, 65], BF, tag="v", name="v")
                        v_sb.setdefault(seg, [None] * 4)[tt] = vt
                        # ones column per head: PV row 64 accumulates the
                        # softmax denominator for free
                        nc.vector.memset(vt[:, :, 64:65], 1.0)
                    vt = v_sb[seg][tt]
                    ps = psA.tile([128, M], FP32, tag="psA", name="ps")
                    for ct in range(CT):
                        mm("qkv_v",
                            ps[:],
                            lhsT=x_sb[seg][:, ct * M + tt * 128:ct * M + (tt + 1) * 128],
                            rhs=wv_sb[:, ct * C + nf * 512:ct * C + (nf + 1) * 512],
                            start=(ct == 0), stop=(ct == CT - 1))
                    nc.scalar.copy(
                        out=vt[:, nf * 8:(nf + 1) * 8, 0:64],
                        in_=ps[:].rearrange("p (h e) -> p h e", e=64))

            def emit_scores(seg, h):
                # scoresT blocks [k, q]: lhsT = k-chunk, rhs = q (no
                # transposes needed anywhere; pT = exp(scoresT) directly)
                et, row = h // 2, (h % 2) * 64
                qh = qk_sb[seg][et][row:row + 64, :]
                kh = qk_sb[seg][8 + et][row:row + 64, :]
                pt_sb = []
                for kc in range(4):
                    n2 = (4 - kc) * 128
                    ps = psS.tile([128, M], FP32, tag="psS", name="ps")
                    mm("scores",
                        ps[:, :n2],
                        lhsT=kh[:, kc * 128:(kc + 1) * 128],
                        rhs=qh[:, kc * 128:], start=True, stop=True)
                    ptk = ptpool.tile([128, M], BF, tag="pt", name="pt")
                    nc.scalar.activation(
                        out=ptk[:, :n2], in_=ps[:, :n2],
                        func=mybir.ActivationFunctionType.Exp)
                    # causal mask: zero the lower triangle of the diagonal
                    # block (keep where q_local >= k_local) on idle GpSimd
                    nc.gpsimd.affine_select(
                        out=ptk[:, 0:128], in_=ptk[:, 0:128],
                        compare_op=mybir.AluOpType.is_ge,
                        fill=0.0, base=0,
                        pattern=[[1, 128]], channel_multiplier=-1)
                    pt_sb.append(ptk)
                return pt_sb

            def emit_pv(seg, h, po, pt_sb):
                # po [65, M]: rows 0:64 = unnormalized outT, row 64 = denom
                for kc in range(4):
                    n2 = (4 - kc) * 128
                    mm("pv",
                        po[:, kc * 128:],
                        lhsT=v_sb[seg][kc][:, h, :],
                        rhs=pt_sb[kc][:, :n2],
                        start=(kc == 0), stop=(kc == 3))
                denrow = small.tile([1, M], FP32, tag="denrow", name="denrow", bufs=3)
                nc.vector.tensor_copy(out=denrow[:], in_=po[64:65, :])
                rdenT = small.tile([1, M], FP32, tag="rdenT", name="rdenT", bufs=3)
                nc.vector.reciprocal_approx_fast(out=rdenT[:], in_=denrow[:])
                rb = rbpool.tile([64, M], FP32, tag="rb", name="rb")
                nc.gpsimd.partition_broadcast(rb[:], rdenT[:], channels=64)
                row = (h % 2) * 64
                nc.vector.tensor_mul(
                    out=oT_sb[seg][h // 2][row:row + 64, :],
                    in0=po[0:64, :], in1=rb[:])

            def emit_proj_tile(seg, ot):
                base = seg * C
                ps = psA.tile([128, M], FP32, tag="psA", name="ps")
                for ct in range(CT):
                    mm("proj",
                        ps[:],
                        lhsT=wout_sb[:, ct * C + ot * 128:ct * C + (ot + 1) * 128],
                        rhs=oT_sb[seg][ct][:],
                        start=(ct == 0), stop=(ct == CT - 1))
                yt = ypool.tile([128, M], FP32, tag="y", name="yt")
                nc.vector.tensor_scalar_add(yt[:], ps[:], bout_sb[:, ot:ot + 1])
                nc.sync.dma_start(
                    out=yT[base + ot * 128:base + (ot + 1) * 128, :], in_=yt[:])

            def emit_attn(seg, filler, warm=None):
                # two-stage software pipeline over heads: scoresT+exp of head
                # h+1 is emitted before PV(h), covering softmax latency.
                # `warm` carries heads whose scores were pre-emitted into the
                # preceding dense stream (pipeline warm-up).
                oT_sb[seg] = [otpool.tile([128, M], BF, tag="ot", name="ot")
                              for _ in range(8)]
                prev = None
                for h in range(H):
                    if warm and h in warm:
                        cur = (h,) + warm[h]
                    else:
                        cur = (h, emit_scores(seg, h),
                               psO.tile([65, M], FP32, tag="psO", name="po"))
                    if prev is not None:
                        ph, pts, po = prev
                        emit_pv(seg, ph, po, pts)
                    filler()
                    prev = cur
                ph, pts, po = prev
                emit_pv(seg, ph, po, pts)

            emit_x(0)
            emit_w_qk_chunk(0)
            nc.sync.dma_start(out=tri_sb[:], in_=tri[:, :])
            nc.sync.dma_start(out=bqk_sb[:], in_=bqk[:, :])
            nc.sync.dma_start(out=bout_sb[:], in_=bout[:, :])
            emit_w_qk_chunk(1)
            emit_w_v()
            emit_w_qk_chunk(2)
            emit_w_qk_chunk(3)
            emit_x(1)
            emit_w_out()
            # seg1 filler interleaves v into the qk stream (weights are
            # long since resident by then); seg0 keeps v last, since the wv
            # DMA lands after the qk weight chunks
            A_ORDER = [0, 1, 16, 2, 3, 17, 4, 5, 18, 6, 7, 19,
                       8, 9, 20, 10, 11, 21, 12, 13, 22, 14, 15, 23]
            for u in range(20):
                emit_qkv_unit(0, u)
            warm0 = {0: (emit_scores(0, 0),
                         psO.tile([65, M], FP32, tag="psO", name="po"))}
            emit_qkv_unit(0, 20)
            emit_qkv_unit(0, 21)
            warm0[1] = (emit_scores(0, 1),
                        psO.tile([65, M], FP32, tag="psO", name="po"))
            emit_qkv_unit(0, 22)
            emit_qkv_unit(0, 23)

            # B(0) with A(1) spread through; B(1) with C(0) spread through;
            # C(1) as the dense tail.
            qkv1 = iter(A_ORDER)

            def fill_qkv1():
                for _ in range(2):
                    u = next(qkv1, None)
                    if u is not None:
                        emit_qkv_unit(1, u)

            emit_attn(0, fill_qkv1, warm=warm0)
            # warm-start seg1's pipeline the same way: its first two heads'
            # score chains begin while seg0's tail PV work runs on the PE
            warm1 = {0: (emit_scores(1, 0),
                         psO.tile([65, M], FP32, tag="psO", name="po")),
                     1: (emit_scores(1, 1),
                         psO.tile([65, M], FP32, tag="psO", name="po"))}
            proj0 = iter(range(8))
            _pcall = [0]

            def fill_proj0():
                # emit on odd slots so the filler lasts the whole phase
                if _pcall[0] % 2 == 1:
                    ot = next(proj0, None)
                    if ot is not None:
                        emit_proj_tile(0, ot)
                _pcall[0] += 1

            emit_attn(1, fill_proj0, warm=warm1)
            for ot in range(8):
                emit_proj_tile(1, ot)

    nc.finalize()
    return nc


def _prep_inputs(x, w_in, b_in, w_out, b_out):
    x = np.asarray(x, dtype=np.float32)
    w_in = np.asarray(w_in, dtype=np.float32)
    b_in = np.asarray(b_in, dtype=np.float32)
    w_out = np.asarray(w_out, dtype=np.float32)
    b_out = np.asarray(b_out, dtype=np.float32)

    # fold 1/sqrt(dh) into the q rows of w_in / b_in
    w_in_s = w_in.copy()
    b_in_s = b_in.copy()
    w_in_s[:C] *= DH ** -0.5
    b_in_s[:C] *= DH ** -0.5

    w_inT0 = np.ascontiguousarray(w_in_s.T).astype(BF16)
    # permute qk columns into [q_p | k_p] pairs matching the consume order
    w_inT = w_inT0.copy()
    for p in range(8):
        w_inT[:, p * 256:p * 256 + 128] = w_inT0[:, p * 128:(p + 1) * 128]
        w_inT[:, p * 256 + 128:(p + 1) * 256] = \
            w_inT0[:, C + p * 128:C + (p + 1) * 128]
    # repack into ct-major chunk layouts (one DMA per chunk on device)
    wp = w_inT.reshape(8, 128, 3 * C)
    wqk = np.ascontiguousarray(np.concatenate(
        [wp[:, :, c4 * 512:(c4 + 1) * 512].transpose(1, 0, 2).reshape(128, 8 * 512)
         for c4 in range(4)], axis=0))                       # (512, 4096)
    wv = np.ascontiguousarray(
        wp[:, :, 2 * C:].transpose(1, 0, 2).reshape(128, 8 * C))  # (128, 8192)
    w_outT = np.ascontiguousarray(w_out.T).astype(BF16)
    wout = np.ascontiguousarray(
        w_outT.reshape(8, 128, C).transpose(1, 0, 2).reshape(128, 8 * C))
    bqk = np.ascontiguousarray(b_in_s[:2 * C].reshape(16, 128).T, dtype=np.float32)
    # v bias folds exactly into an effective output bias:
    #   (p @ (v + 1 b_v^T)) / denom = (p @ v)/denom + b_v
    b_out_eff = b_out + w_out @ b_in[2 * C:]
    bout = np.ascontiguousarray(b_out_eff.reshape(8, 128).T, dtype=np.float32)

    # dilated gather + transpose + ct-major pack: per-core (2*128, 8*M)
    xi = x.reshape(B, S, W_SEG, C)[:, :, ::RATE, :]        # (B, S, M, C)
    xiT = np.ascontiguousarray(xi.transpose(0, 1, 3, 2)).astype(BF16)  # (B,S,C,M)
    xiT = xiT.reshape(16, 8, 128, M).transpose(0, 2, 1, 3)  # (16,128,8,M)
    xiT = np.ascontiguousarray(xiT).reshape(N_CORES, SEG_PER_CORE * 128, 8 * M)

    i = np.arange(128)[:, None]
    j = np.arange(128)[None, :]
    # scoresT orientation: rows = k, cols = q; valid iff q >= k
    tri = np.where(j >= i, np.float32(0), np.float32(-1e9))

    in_maps = []
    for c in range(N_CORES):
        in_maps.append({
            "xiT": np.ascontiguousarray(xiT[c]),
            "wqk": wqk,
            "wv": wv,
            "wout": wout,
            "bqk": bqk,
            "bout": bout,
            "tri": tri,
        })
    return in_maps


def kernel(x, w_in, b_in, w_out, b_out, _trace=False):
    if "nc" not in _CACHE:
        _CACHE["nc"] = _build()
    nc = _CACHE["nc"]

    in_maps = _prep_inputs(x, w_in, b_in, w_out, b_out)
    res = run_bass_kernel_spmd(
        nc, in_maps, core_ids=list(range(N_CORES)), trace=_trace)
    _CACHE["last_result"] = res

    out = np.zeros((B, N, C), dtype=np.float32)
    ov = out.reshape(B, S, W_SEG, C)
    for c in range(N_CORES):
        yTc = res.results[c]["yT"]                       # (2C, M) fp32
        for seg in range(SEG_PER_CORE):
            gseg = c * SEG_PER_CORE + seg
            b, s = divmod(gseg, S)
            ov[b, s, ::RATE, :] = yTc[seg * C:(seg + 1) * C, :].T
    return out
